# revision 4
# baseline (speedup 1.0000x reference)
"""CRF forward-algorithm kernel for Trainium2 (8 NeuronCores, data-parallel over batch).

Math: the reference computes, per sample b,
    fv_{t+1}[next] = x_t[next] + logsumexp_prev(transit[next, prev] + fv_t[prev])   (t < len_b)
    alpha[b] = logsumexp_i(fv_{len_b}[i] + transit[STOP, i])

In linear space with E = exp(transit) this is
    w_{t+1} = exp(x_t) * (E @ w_t),      fv_t = log(w_t) + c_t
so each timestep is one fp16 128x128 @ 128x32 matmul (PE) plus one elementwise
multiply.  The 512-step chain is serial, so wall time = 512 x round latency
(PE -> PSUM -> multiply -> SBUF -> PE).  The round is minimized by:
  * loading E into the PE array once (ldweights) and marking every step matmul
    non-self-loading, which removes the ~100ns weight reload that otherwise
    sits after the semaphore wait on the critical path;
  * splitting the 32 batch columns into two groups whose multiplies run
    concurrently on the Vector (DVE) and Pool engines, each reading its own
    PSUM bank, so per-round engine time is halved.
exp(x) is pre-scaled by 1/256 and the state is renormalized every K=8 steps:
the normalizer is the fp16 state row 0 (already in SBUF), its reciprocal is
recorded for the host, broadcast across partitions on Pool, and folded into
the exp(x) slice of step tau+D in each engine's own queue order, so the renorm
never adds semaphores to the serial chain.  Because alpha needs
(E @ w_len)[STOP] and STOP = 127 is the last row of E, the per-step capture is
just row 127 of the state; the state lives in two alternating 32-slot rings so
row 127 of a finished ring is DMA'd out with 32 steps of WAR slack.  The final
log/gather bookkeeping (O(B*T) scalar work) runs on host in float64 from the
captures.
"""

import sys

sys.path.insert(0, "/opt/trn_rl_repo")

import numpy as np
from contextlib import ExitStack

import concourse.bass as bass
import concourse.tile as tile
import concourse.mybir as mybir
from concourse import bacc, bass_utils

# Problem constants (hardcoded per contract).
B, T, L = 256, 512, 128
NCORES = 8
BC = B // NCORES          # 32 samples per core
K = 8                     # renormalization period
D = 4                     # renorm application delay (steps after tau)
CAPB = 32                 # capture block (ring size)
CH = 128                  # x chunk length in timesteps
NCAP = T // CAPB          # capture blocks
NNORM = 63                # norms m=0..62: tau=8(m+1)<=504, applied at tau+D
G = 2                     # batch groups: g0 -> Vector, g1 -> Pool
GS = BC // G
SCALE_LN = float(np.log(256.0))
F32 = mybir.dt.float32
DT = mybir.dt.float16     # state/weights dtype

_CACHED_NC = None


def _build_bass():
    """Build the single-core Bass program (shared SPMD across 8 cores)."""
    nc = bacc.Bacc("TRN2", debug=False)

    xT = nc.dram_tensor("xT", [L, T * BC], F32, kind="ExternalInput").ap()
    trT = nc.dram_tensor("trT", [L, L], F32, kind="ExternalInput").ap()
    # hist[j*CAPB*BC + s*BC + b] = w_{32j+1+s}[127, b]; tail BC entries are
    # (E @ w_512)[127].
    hist = nc.dram_tensor("hist", [1, T * BC + BC], DT, kind="ExternalOutput").ap()
    rhist = nc.dram_tensor("rhist", [1, NNORM * BC], DT, kind="ExternalOutput").ap()

    with tile.TileContext(nc) as tc, ExitStack() as ctx, \
            nc.allow_low_precision(reason="fp16 state validated against f64 ref"):
        const_pool = ctx.enter_context(tc.tile_pool(name="const", bufs=1))
        state_pool = ctx.enter_context(tc.tile_pool(name="state", bufs=1))
        xin_pool = ctx.enter_context(tc.tile_pool(name="xin", bufs=2))
        ex_pool = ctx.enter_context(tc.tile_pool(name="ex", bufs=3))
        ps_pool = ctx.enter_context(tc.tile_pool(name="ps", bufs=4, space="PSUM"))

        # Constants.
        nbias = const_pool.tile([L, 1], F32)
        nc.vector.memset(nbias[:], -SCALE_LN)
        tr_sb = const_pool.tile([L, L], F32)
        nc.sync.dma_start(tr_sb[:], trT[:, :])
        E_sb = const_pool.tile([L, L], DT)
        nc.scalar.activation(E_sb[:], tr_sb[:], mybir.ActivationFunctionType.Exp)
        # E stays resident in the PE array for the whole chain; every step
        # matmul below is marked non-self-loading.
        nc.tensor.ldweights(E_sb[:])

        # Reciprocal history (one fp16 reciprocal per norm per sample).
        rh_sb = state_pool.tile([1, NNORM * BC], DT)

        # Two state rings: ring(j) = j%2 holds w_{32j+1..32j+32} in slots 0..31.
        WA = state_pool.tile([L, CAPB * BC], DT)
        WB = state_pool.tile([L, CAPB * BC], DT)
        rings = [WA, WB]
        # w_0 = onehot(START=0) lives at ring 1, slot 31.
        nc.vector.memset(WB[:, (CAPB - 1) * BC:CAPB * BC], 0.0)
        nc.vector.memset(WB[0:1, (CAPB - 1) * BC:CAPB * BC], 1.0)

        def wslot(t):
            """AP of w_t (full BC columns)."""
            ring = rings[((t - 1) // CAPB) % 2]
            s = (t - 1) % CAPB
            return ring[:, s * BC:(s + 1) * BC]

        rbc_pool = ctx.enter_context(tc.tile_pool(name="rbc", bufs=2))

        ex_tiles = {}   # granule index -> ex tile (CAPB steps each)
        pend_R = None   # (broadcast reciprocal sbuf tile, application step)
        # Pool cannot read PSUM on TRN2, so both groups' multiplies run on
        # DVE (in-order, so g0/g1 need no semaphores between them); Pool
        # handles the renorm broadcast (SBUF-only) off the chain.
        mults = [nc.vector, nc.vector]

        # First chunk split small so step 0 starts ~9us earlier.
        chunk_steps = [CAPB, CH - CAPB] + [CH] * (T // CH - 1)
        chunk_t0 = np.cumsum([0] + chunk_steps[:-1]).tolist()
        for cs, ct0 in zip(chunk_steps, chunk_t0):
            xt = xin_pool.tile([L, cs * BC], F32, tag="xt")
            nc.sync.dma_start(xt[:], xT[:, ct0 * BC:(ct0 + cs) * BC])
            for jj in range(cs // CAPB):
                j = (ct0 // CAPB) + jj      # capture block index
                jo = jj                      # granule offset within chunk
                ex = ex_pool.tile([L, CAPB * BC], DT)
                nc.scalar.activation(
                    ex[:], xt[:, jo * CAPB * BC:(jo + 1) * CAPB * BC],
                    mybir.ActivationFunctionType.Exp, bias=nbias[:],
                )
                ex_tiles[j] = ex
                for i in range(CAPB):
                    t = j * CAPB + i
                    # Apply a pending renorm to this step's ex slice, each
                    # engine scaling the half it will consume (same-queue
                    # ordering, no extra semaphores on the chain).
                    if pend_R is not None and pend_R[1] == t:
                        R = pend_R[0]
                        for g in range(G):
                            sl = slice(i * BC + g * GS, i * BC + (g + 1) * GS)
                            mults[g].tensor_mul(ex[:, sl], ex[:, sl],
                                                R[:, g * GS:(g + 1) * GS])
                        pend_R = None
                    src = wslot(t)
                    dst = wslot(t + 1)
                    for g in range(G):
                        P = ps_pool.tile([L, GS], F32, tag=f"P{g}")
                        mm = nc.tensor.matmul(P[:], E_sb[:],
                                              src[:, g * GS:(g + 1) * GS],
                                              start=True, stop=True)
                        mm.ins.ldweights = False
                        mults[g].tensor_mul(dst[:, g * GS:(g + 1) * GS],
                                            ex[:, i * BC + g * GS:
                                               i * BC + (g + 1) * GS],
                                            P[:])
                    # Renorm trigger: tau = t = K(m+1); normalizer = the fp16
                    # state row 0 just written (any per-column scale works;
                    # the host uses the recorded fp16 reciprocal exactly).
                    # Broadcast it on Pool and fold into the ex slice of step
                    # tau+D, off the serial matmul/multiply chain.
                    if t % K == 0 and t > 0 and t + D <= T - 1:
                        m = t // K - 1
                        nc.vector.reciprocal(rh_sb[0:1, m * BC:(m + 1) * BC],
                                             dst[0:1, :])
                        Rbc = rbc_pool.tile([L, BC], DT)
                        nc.gpsimd.partition_broadcast(
                            Rbc[:], rh_sb[0:1, m * BC:(m + 1) * BC])
                        pend_R = (Rbc, t + D)
                # Capture row 127 of the finished ring (w_{32j+1..32j+32});
                # the double ring gives this DMA 32 steps of WAR slack.
                ring = rings[j % 2]
                nc.sync.dma_start(
                    hist[0:1, j * CAPB * BC:(j + 1) * CAPB * BC],
                    ring[127:128, :])
                if j - 2 in ex_tiles:
                    del ex_tiles[j - 2]

        # Final (E @ w_512)[127] for samples with len == T.
        Pf = ps_pool.tile([L, BC], F32, tag="P0")
        nc.tensor.matmul(Pf[:], E_sb[:], wslot(T), start=True, stop=True)
        capf = state_pool.tile([L, BC], DT)
        nc.vector.tensor_copy(capf[:], Pf[:])
        nc.sync.dma_start(hist[0:1, T * BC:T * BC + BC], capf[127:128, :])
        nc.sync.dma_start(rhist[0:1, :], rh_sb[:])

    nc.compile()
    return nc


def _get_nc():
    global _CACHED_NC
    if _CACHED_NC is None:
        _CACHED_NC = _build_bass()
    return _CACHED_NC


def run_on_device(x, transit_matrix, **spmd_kwargs):
    """Shard inputs, run the SPMD kernel on 8 cores, return per-core results."""
    xT = np.ascontiguousarray(np.asarray(x, np.float32).transpose(2, 1, 0))
    trT = np.ascontiguousarray(np.asarray(transit_matrix, np.float32).T)
    in_maps = []
    for c in range(NCORES):
        xc = np.ascontiguousarray(xT[:, :, c * BC:(c + 1) * BC]).reshape(L, T * BC)
        in_maps.append({"xT": xc, "trT": trT})
    nc = _get_nc()
    return bass_utils.run_bass_kernel_spmd(
        nc, in_maps, core_ids=list(range(NCORES)), **spmd_kwargs
    )


def finish_on_host(results, x, lengths):
    """Reconstruct alpha[b] in float64 from the device captures.

    fv_t = ln(w_t) + t*SCALE_LN + sum of ln(s_m) over norms applied before t
    (norm m: s_m = 1/r_m, r_m recorded; applied at step a_m = K(m+1)+D).
    For len < T the capture is w_{len+1}[127] = exp(x[b,len,127])/256 *
    (E @ w_len)[127] (with the step-len renorm folded in when a_m == len), which
    collapses to the uniform formula below; for len == T the tail capture is
    (E @ w_512)[127] directly.
    """
    lengths = np.asarray(lengths).astype(np.int64)
    x = np.asarray(x)
    alpha = np.empty(B, np.float64)
    for c in range(NCORES):
        hist = results[c]["hist"].reshape(-1).astype(np.float64)
        rh = results[c]["rhist"].reshape(-1).astype(np.float64)
        lnS = -np.log(rh.reshape(NNORM, BC))          # ln s_m per norm m
        cum = np.zeros((NNORM + 1, BC))
        cum[1:] = np.cumsum(lnS, axis=0)
        hist_blk = hist[:T * BC].reshape(T, BC)       # hist_blk[t-1] = w_t[127]
        cap512 = hist[T * BC:]

        ln = lengths[c * BC:(c + 1) * BC]             # (BC,)
        bi = np.arange(BC)
        full = ln == T
        nf = ~full
        out = np.empty(BC, np.float64)
        # Captures that underflowed deep into fp16 subnormals lose log
        # accuracy; flag them (NaN) for the exact host fallback in kernel().
        with np.errstate(divide="ignore", invalid="ignore"):
            out[full] = T * SCALE_LN + cum[NNORM, bi[full]] + np.log(
                np.where(cap512[full] < 3e-7, np.nan, cap512[full]))
        cap = hist_blk[ln[nf], bi[nf]]                # w_{len+1}[127]
        cap = np.where(cap < 3e-7, np.nan, cap)
        x127 = x[c * BC + bi[nf], ln[nf], 127].astype(np.float64)
        # norms applied at a_m = K(m+1)+D <= len: count = (len-D)//K, clipped
        nidx = np.clip((ln[nf] - D) // K, 0, NNORM)
        with np.errstate(divide="ignore", invalid="ignore"):
            out[nf] = (np.log(cap) - x127 + (ln[nf] + 1) * SCALE_LN
                       + cum[nidx, bi[nf]])
        alpha[c * BC:(c + 1) * BC] = out
    return alpha.astype(np.float32)


def _crf_alpha_single(xb, tr, length):
    """Exact single-sample CRF forward in float64 (rare-fallback path)."""
    NEG = -10000.0
    trd = np.asarray(tr, np.float64)
    fv = np.full(L, NEG)
    fv[0] = 0.0
    for t in range(int(length)):
        sc = trd + fv[None, :] + np.asarray(xb[t], np.float64)[:, None]
        m = sc.max(axis=1)
        fv = m + np.log(np.exp(sc - m[:, None]).sum(axis=1))
    term = fv + trd[L - 1]
    m = term.max()
    return m + np.log(np.exp(term - m).sum())


def kernel(x, transit_matrix, lengths):
    x = np.asarray(x, np.float32)
    assert x.shape == (B, T, L), x.shape
    res = run_on_device(x, transit_matrix)
    alpha = finish_on_host(res.results, x, lengths)
    # fp16 captures can in principle underflow to subnormal/zero for extreme
    # samples; recompute those few (if any) exactly on host.
    bad = ~np.isfinite(alpha)
    if bad.any():
        ln = np.asarray(lengths).astype(np.int64)
        for b in np.nonzero(bad)[0]:
            alpha[b] = _crf_alpha_single(x[b], transit_matrix, ln[b])
    return alpha


# revision 13
# speedup vs baseline: 1.0037x; 1.0037x over previous
"""CRF forward-algorithm kernel for Trainium2 (8 NeuronCores, data-parallel over batch).

Math: the reference computes, per sample b,
    fv_{t+1}[next] = x_t[next] + logsumexp_prev(transit[next, prev] + fv_t[prev])   (t < len_b)
    alpha[b] = logsumexp_i(fv_{len_b}[i] + transit[STOP, i])

In linear space with E = exp(transit) this is
    w_{t+1} = exp(x_t) * (E @ w_t),      fv_t = log(w_t) + c_t
so each timestep is one fp16 128x128 @ 128x32 matmul (PE) plus one elementwise
multiply.  The 512-step chain is serial, so wall time = 512 x round latency
(PE -> PSUM -> multiply -> SBUF -> PE).  The round is minimized by:
  * loading E into the PE array once (ldweights) and marking every step matmul
    non-self-loading, which removes the ~100ns weight reload that otherwise
    sits after the semaphore wait on the critical path;
  * splitting the 32 batch columns into two groups whose multiplies run
    concurrently on the Vector (DVE) and Pool engines, each reading its own
    PSUM bank, so per-round engine time is halved.
exp(x) is pre-scaled by 1/256 and the state is renormalized every K=8 steps:
the normalizer is the fp16 state row 0 (already in SBUF), its reciprocal is
recorded for the host, broadcast across partitions on Pool, and folded into
the exp(x) slice of step tau+D in each engine's own queue order, so the renorm
never adds semaphores to the serial chain.  Because alpha needs
(E @ w_len)[STOP] and STOP = 127 is the last row of E, the per-step capture is
just row 127 of the state; the state lives in two alternating 32-slot rings so
row 127 of a finished ring is DMA'd out with 32 steps of WAR slack.  The final
log/gather bookkeeping (O(B*T) scalar work) runs on host in float64 from the
captures.
"""

import sys

sys.path.insert(0, "/opt/trn_rl_repo")

import numpy as np
from contextlib import ExitStack

import concourse.bass as bass
import concourse.tile as tile
import concourse.mybir as mybir
from concourse import bacc, bass_utils



# Problem constants (hardcoded per contract).
B, T, L = 256, 512, 128
NCORES = 8
BC = B // NCORES          # 32 samples per core
K = 8                     # renormalization period
D = 4                     # renorm application delay (steps after tau)
CAPB = 32                 # capture block (ring size)
CH = 128                  # x chunk length in timesteps
NCAP = T // CAPB          # capture blocks
NNORM = 63                # norms m=0..62: tau=8(m+1)<=504, applied at tau+D
G = 2                     # batch groups: g0 -> Vector, g1 -> Pool
GS = BC // G
SCALE_LN = float(np.log(256.0))
F32 = mybir.dt.float32
DT = mybir.dt.float16     # state/weights dtype

_CACHED_NC = None


def _build_bass():
    """Build the single-core Bass program (shared SPMD across 8 cores)."""
    nc = bacc.Bacc("TRN2", debug=False)

    xT = nc.dram_tensor("xT", [L, T * BC], F32, kind="ExternalInput").ap()
    trT = nc.dram_tensor("trT", [L, L], F32, kind="ExternalInput").ap()
    # hist[j*CAPB*BC + s*BC + b] = w_{32j+1+s}[127, b]; tail BC entries are
    # (E @ w_512)[127].
    hist = nc.dram_tensor("hist", [1, T * BC + BC], DT, kind="ExternalOutput").ap()
    rhist = nc.dram_tensor("rhist", [1, NNORM * BC], DT, kind="ExternalOutput").ap()

    keep_ld_names = set()
    with tile.TileContext(nc) as tc, ExitStack() as ctx, \
            nc.allow_low_precision(reason="fp16 state validated against f64 ref"):
        const_pool = ctx.enter_context(tc.tile_pool(name="const", bufs=1))
        state_pool = ctx.enter_context(tc.tile_pool(name="state", bufs=1))
        xin_pool = ctx.enter_context(tc.tile_pool(name="xin", bufs=2))
        ex_pool = ctx.enter_context(tc.tile_pool(name="ex", bufs=3))
        ps_pool = ctx.enter_context(tc.tile_pool(name="ps", bufs=4, space="PSUM"))

        # Constants.
        nbias = const_pool.tile([L, 1], F32)
        nc.vector.memset(nbias[:], -SCALE_LN)
        tr_sb = const_pool.tile([L, L], F32)
        nc.sync.dma_start(tr_sb[:], trT[:, :])
        E_sb = const_pool.tile([L, L], DT)
        nc.scalar.activation(E_sb[:], tr_sb[:], mybir.ActivationFunctionType.Exp)
        # E is loaded into the PE array exactly once and stays resident for
        # the whole chain: every matmul below is flagged non-self-loading and
        # the redundant per-matmul InstLdweights that Tile re-inserts are
        # deleted from the module after the TileContext exits (they carry no
        # semaphore waits, so removal is sync-safe).  This takes the ~100ns
        # 128-row weight reload off the serial matmul->multiply chain.
        lw = nc.tensor.ldweights(E_sb[:])
        keep_ld_names.add(lw.ins.name)

        # Reciprocal history (one fp16 reciprocal per norm per sample).
        rh_sb = state_pool.tile([1, NNORM * BC], DT)

        # Two state rings: ring(j) = j%2 holds w_{32j+1..32j+32} in slots 0..31.
        WA = state_pool.tile([L, CAPB * BC], DT)
        WB = state_pool.tile([L, CAPB * BC], DT)
        rings = [WA, WB]
        # w_0 = onehot(START=0) lives at ring 1, slot 31.
        nc.vector.memset(WB[:, (CAPB - 1) * BC:CAPB * BC], 0.0)
        nc.vector.memset(WB[0:1, (CAPB - 1) * BC:CAPB * BC], 1.0)

        def wslot(t):
            """AP of w_t (full BC columns)."""
            ring = rings[((t - 1) // CAPB) % 2]
            s = (t - 1) % CAPB
            return ring[:, s * BC:(s + 1) * BC]

        rbc_pool = ctx.enter_context(tc.tile_pool(name="rbc", bufs=2))

        ex_tiles = {}   # granule index -> ex tile (CAPB steps each)
        pend_R = None   # (broadcast reciprocal sbuf tile, application step)
        # Pool cannot read PSUM on TRN2, so both groups' multiplies run on
        # DVE (in-order, so g0/g1 need no semaphores between them); Pool
        # handles the renorm broadcast (SBUF-only) off the chain.
        mults = [nc.vector, nc.vector]

        # First chunk split small so step 0 starts ~9us earlier.
        chunk_steps = [CAPB, CH - CAPB] + [CH] * (T // CH - 1)
        chunk_t0 = np.cumsum([0] + chunk_steps[:-1]).tolist()
        for cs, ct0 in zip(chunk_steps, chunk_t0):
            xt = xin_pool.tile([L, cs * BC], F32, tag="xt")
            nc.sync.dma_start(xt[:], xT[:, ct0 * BC:(ct0 + cs) * BC])
            for jj in range(cs // CAPB):
                j = (ct0 // CAPB) + jj      # capture block index
                jo = jj                      # granule offset within chunk
                ex = ex_pool.tile([L, CAPB * BC], DT)
                nc.scalar.activation(
                    ex[:], xt[:, jo * CAPB * BC:(jo + 1) * CAPB * BC],
                    mybir.ActivationFunctionType.Exp, bias=nbias[:],
                )
                ex_tiles[j] = ex
                for i in range(CAPB):
                    t = j * CAPB + i
                    # Apply a pending renorm to this step's ex slice, each
                    # engine scaling the half it will consume (same-queue
                    # ordering, no extra semaphores on the chain).
                    if pend_R is not None and pend_R[1] == t:
                        R = pend_R[0]
                        for g in range(G):
                            sl = slice(i * BC + g * GS, i * BC + (g + 1) * GS)
                            mults[g].tensor_mul(ex[:, sl], ex[:, sl],
                                                R[:, g * GS:(g + 1) * GS])
                        pend_R = None
                    src = wslot(t)
                    dst = wslot(t + 1)
                    for g in range(G):
                        P = ps_pool.tile([L, GS], F32, tag=f"P{g}")
                        mm = nc.tensor.matmul(P[:], E_sb[:],
                                              src[:, g * GS:(g + 1) * GS],
                                              start=True, stop=True)
                        mm.ins.ldweights = False
                        mults[g].tensor_mul(dst[:, g * GS:(g + 1) * GS],
                                            ex[:, i * BC + g * GS:
                                               i * BC + (g + 1) * GS],
                                            P[:])
                    # Renorm trigger: tau = t = K(m+1); normalizer = the fp16
                    # state row 0 just written (any per-column scale works;
                    # the host uses the recorded fp16 reciprocal exactly).
                    # Broadcast it on Pool and fold into the ex slice of step
                    # tau+D, off the serial matmul/multiply chain.
                    if t % K == 0 and t > 0 and t + D <= T - 1:
                        m = t // K - 1
                        nc.vector.reciprocal(rh_sb[0:1, m * BC:(m + 1) * BC],
                                             dst[0:1, :])
                        Rbc = rbc_pool.tile([L, BC], DT)
                        nc.gpsimd.partition_broadcast(
                            Rbc[:], rh_sb[0:1, m * BC:(m + 1) * BC])
                        pend_R = (Rbc, t + D)
                # Capture row 127 of the finished ring (w_{32j+1..32j+32});
                # the double ring gives this DMA 32 steps of WAR slack.
                ring = rings[j % 2]
                nc.sync.dma_start(
                    hist[0:1, j * CAPB * BC:(j + 1) * CAPB * BC],
                    ring[127:128, :])
                if j - 2 in ex_tiles:
                    del ex_tiles[j - 2]

        # Final (E @ w_512)[127] for samples with len == T.
        Pf = ps_pool.tile([L, BC], F32, tag="P0")
        mmf = nc.tensor.matmul(Pf[:], E_sb[:], wslot(T), start=True, stop=True)
        mmf.ins.ldweights = False
        capf = state_pool.tile([L, BC], DT)
        nc.vector.tensor_copy(capf[:], Pf[:])
        nc.sync.dma_start(hist[0:1, T * BC:T * BC + BC], capf[127:128, :])
        nc.sync.dma_start(rhist[0:1, :], rh_sb[:])

    # Tile re-pairs every matmul with its own InstLdweights regardless of the
    # non-self-loading flag; strip those (keeping the single explicit load).
    # They carry no semaphore waits (verified: all waits live on the matmuls),
    # so deleting them does not disturb synchronization.
    removed = 0
    for fn in nc.m.functions:
        for bb in fn.blocks:
            insts = bb.instructions
            for i in range(len(insts) - 1, -1, -1):
                inst = insts[i]
                if (type(inst).__name__ == "InstLdweights"
                        and inst.name not in keep_ld_names):
                    si = inst.sync_info
                    assert si is None or (not si.on_wait and not si.on_update), \
                        f"ldweights {inst.name} carries sync; refusing to drop"
                    del insts[i]
                    removed += 1
    assert removed == 1025, f"expected 1025 redundant ldweights, got {removed}"

    nc.compile()
    return nc


def _get_nc():
    global _CACHED_NC
    if _CACHED_NC is None:
        _CACHED_NC = _build_bass()
    return _CACHED_NC


def run_on_device(x, transit_matrix, **spmd_kwargs):
    """Shard inputs, run the SPMD kernel on 8 cores, return per-core results."""
    xT = np.ascontiguousarray(np.asarray(x, np.float32).transpose(2, 1, 0))
    trT = np.ascontiguousarray(np.asarray(transit_matrix, np.float32).T)
    in_maps = []
    for c in range(NCORES):
        xc = np.ascontiguousarray(xT[:, :, c * BC:(c + 1) * BC]).reshape(L, T * BC)
        in_maps.append({"xT": xc, "trT": trT})
    nc = _get_nc()
    return bass_utils.run_bass_kernel_spmd(
        nc, in_maps, core_ids=list(range(NCORES)), **spmd_kwargs
    )


def finish_on_host(results, x, lengths):
    """Reconstruct alpha[b] in float64 from the device captures.

    fv_t = ln(w_t) + t*SCALE_LN + sum of ln(s_m) over norms applied before t
    (norm m: s_m = 1/r_m, r_m recorded; applied at step a_m = K(m+1)+D).
    For len < T the capture is w_{len+1}[127] = exp(x[b,len,127])/256 *
    (E @ w_len)[127] (with the step-len renorm folded in when a_m == len), which
    collapses to the uniform formula below; for len == T the tail capture is
    (E @ w_512)[127] directly.
    """
    lengths = np.asarray(lengths).astype(np.int64)
    x = np.asarray(x)
    alpha = np.empty(B, np.float64)
    for c in range(NCORES):
        hist = results[c]["hist"].reshape(-1).astype(np.float64)
        rh = results[c]["rhist"].reshape(-1).astype(np.float64)
        lnS = -np.log(rh.reshape(NNORM, BC))          # ln s_m per norm m
        cum = np.zeros((NNORM + 1, BC))
        cum[1:] = np.cumsum(lnS, axis=0)
        hist_blk = hist[:T * BC].reshape(T, BC)       # hist_blk[t-1] = w_t[127]
        cap512 = hist[T * BC:]

        ln = lengths[c * BC:(c + 1) * BC]             # (BC,)
        bi = np.arange(BC)
        full = ln == T
        nf = ~full
        out = np.empty(BC, np.float64)
        # Captures that underflowed deep into fp16 subnormals lose log
        # accuracy; flag them (NaN) for the exact host fallback in kernel().
        with np.errstate(divide="ignore", invalid="ignore"):
            out[full] = T * SCALE_LN + cum[NNORM, bi[full]] + np.log(
                np.where(cap512[full] < 3e-7, np.nan, cap512[full]))
        cap = hist_blk[ln[nf], bi[nf]]                # w_{len+1}[127]
        cap = np.where(cap < 3e-7, np.nan, cap)
        x127 = x[c * BC + bi[nf], ln[nf], 127].astype(np.float64)
        # norms applied at a_m = K(m+1)+D <= len: count = (len-D)//K, clipped
        nidx = np.clip((ln[nf] - D) // K, 0, NNORM)
        with np.errstate(divide="ignore", invalid="ignore"):
            out[nf] = (np.log(cap) - x127 + (ln[nf] + 1) * SCALE_LN
                       + cum[nidx, bi[nf]])
        alpha[c * BC:(c + 1) * BC] = out
    return alpha.astype(np.float32)


def _crf_alpha_single(xb, tr, length):
    """Exact single-sample CRF forward in float64 (rare-fallback path)."""
    NEG = -10000.0
    trd = np.asarray(tr, np.float64)
    fv = np.full(L, NEG)
    fv[0] = 0.0
    for t in range(int(length)):
        sc = trd + fv[None, :] + np.asarray(xb[t], np.float64)[:, None]
        m = sc.max(axis=1)
        fv = m + np.log(np.exp(sc - m[:, None]).sum(axis=1))
    term = fv + trd[L - 1]
    m = term.max()
    return m + np.log(np.exp(term - m).sum())


def kernel(x, transit_matrix, lengths):
    x = np.asarray(x, np.float32)
    assert x.shape == (B, T, L), x.shape
    res = run_on_device(x, transit_matrix)
    alpha = finish_on_host(res.results, x, lengths)
    # fp16 captures can in principle underflow to subnormal/zero for extreme
    # samples; recompute those few (if any) exactly on host.
    bad = ~np.isfinite(alpha)
    if bad.any():
        ln = np.asarray(lengths).astype(np.int64)
        for b in np.nonzero(bad)[0]:
            alpha[b] = _crf_alpha_single(x[b], transit_matrix, ln[b])
    return alpha


# revision 25
# speedup vs baseline: 1.0056x; 1.0019x over previous
"""CRF forward-algorithm kernel for Trainium2 (8 NeuronCores, data-parallel over batch).

Math: the reference computes, per sample b,
    fv_{t+1}[next] = x_t[next] + logsumexp_prev(transit[next, prev] + fv_t[prev])   (t < len_b)
    alpha[b] = logsumexp_i(fv_{len_b}[i] + transit[STOP, i])

In linear space with E = exp(transit) this is
    w_{t+1} = exp(x_t) * (E @ w_t),      fv_t = log(w_t) + c_t
so each timestep is one fp16 128x128 @ 128x32 matmul (PE) plus one elementwise
multiply.  The 512-step chain is serial, so wall time = 512 x round latency
(PE -> PSUM -> multiply -> SBUF -> PE).  The round is minimized by:
  * loading E into the PE array once (ldweights) and marking every step matmul
    non-self-loading, which removes the ~100ns weight reload that otherwise
    sits after the semaphore wait on the critical path;
  * splitting the 32 batch columns into two groups whose multiplies run
    concurrently on the Vector (DVE) and Pool engines, each reading its own
    PSUM bank, so per-round engine time is halved.
exp(x) is pre-scaled by 1/256 and the state is renormalized every K=8 steps:
the normalizer is the fp16 state row 0 (already in SBUF), its reciprocal is
recorded for the host, broadcast across partitions on Pool, and folded into
the exp(x) slice of step tau+D in each engine's own queue order, so the renorm
never adds semaphores to the serial chain.  Because alpha needs
(E @ w_len)[STOP] and STOP = 127 is the last row of E, the per-step capture is
just row 127 of the state; the state lives in two alternating 32-slot rings so
row 127 of a finished ring is DMA'd out with 32 steps of WAR slack.  The final
log/gather bookkeeping (O(B*T) scalar work) runs on host in float64 from the
captures.
"""

import sys

sys.path.insert(0, "/opt/trn_rl_repo")

import numpy as np
from contextlib import ExitStack

import concourse.bass as bass
import concourse.tile as tile
import concourse.mybir as mybir
from concourse import bacc, bass_utils



# Problem constants (hardcoded per contract).
B, T, L = 256, 512, 128
NCORES = 8
BC = B // NCORES          # 32 samples per core
K = 8                     # renormalization period
D = 4                     # renorm application delay (steps after tau)
CAPB = 32                 # capture block (ring size)
CH = 128                  # x chunk length in timesteps
NCAP = T // CAPB          # capture blocks
NNORM = 63                # norms m=0..62: tau=8(m+1)<=504, applied at tau+D
G = 2                     # batch groups: g0 -> Vector, g1 -> Pool
GS = BC // G
SCALE_LN = float(np.log(256.0))
F32 = mybir.dt.float32
DT = mybir.dt.float16     # state/weights dtype

_CACHED_NC = None





def _build_bass():
    """Build the single-core Bass program (shared SPMD across 8 cores)."""
    nc = bacc.Bacc("TRN2", debug=False)

    xT = nc.dram_tensor("xT", [L, T * BC], F32, kind="ExternalInput").ap()
    trT = nc.dram_tensor("trT", [L, L], F32, kind="ExternalInput").ap()
    # hist[j*CAPB*BC + s*BC + b] = w_{32j+1+s}[127, b]; tail BC entries are
    # (E @ w_512)[127].
    hist = nc.dram_tensor("hist", [1, T * BC + BC], DT, kind="ExternalOutput").ap()
    rhist = nc.dram_tensor("rhist", [1, NNORM * BC], DT, kind="ExternalOutput").ap()

    keep_ld_names = set()
    with tile.TileContext(nc) as tc, ExitStack() as ctx, \
            nc.allow_low_precision(reason="fp16 state validated against f64 ref"):
        const_pool = ctx.enter_context(tc.tile_pool(name="const", bufs=1))
        state_pool = ctx.enter_context(tc.tile_pool(name="state", bufs=1))
        xin_pool = ctx.enter_context(tc.tile_pool(name="xin", bufs=2))
        ex_pool = ctx.enter_context(tc.tile_pool(name="ex", bufs=3))
        ps_pool = ctx.enter_context(tc.tile_pool(name="ps", bufs=4, space="PSUM"))

        # Constants.
        nbias = const_pool.tile([L, 1], F32)
        nc.vector.memset(nbias[:], -SCALE_LN)

        tr_sb = const_pool.tile([L, L], F32)
        nc.sync.dma_start(tr_sb[:], trT[:, :])
        E_sb = const_pool.tile([L, L], DT)
        nc.scalar.activation(E_sb[:], tr_sb[:], mybir.ActivationFunctionType.Exp)
        # E is loaded into the PE array exactly once and stays resident for
        # the whole chain: every matmul below is flagged non-self-loading and
        # the redundant per-matmul InstLdweights that Tile re-inserts are
        # deleted from the module after the TileContext exits (they carry no
        # semaphore waits, so removal is sync-safe).  This takes the ~100ns
        # 128-row weight reload off the serial matmul->multiply chain.
        lw = nc.tensor.ldweights(E_sb[:])
        keep_ld_names.add(lw.ins.name)

        # Reciprocal history (one fp16 reciprocal per norm per sample).
        rh_sb = state_pool.tile([1, NNORM * BC], DT)

        # Two state rings: ring(j) = j%2 holds w_{32j+1..32j+32} in slots 0..31.
        WA = state_pool.tile([L, CAPB * BC], DT)
        WB = state_pool.tile([L, CAPB * BC], DT)
        rings = [WA, WB]
        # w_0 = onehot(START=0) lives at ring 1, slot 31.
        nc.vector.memset(WB[:, (CAPB - 1) * BC:CAPB * BC], 0.0)
        nc.vector.memset(WB[0:1, (CAPB - 1) * BC:CAPB * BC], 1.0)

        def wslot(t):
            """AP of w_t (full BC columns)."""
            ring = rings[((t - 1) // CAPB) % 2]
            s = (t - 1) % CAPB
            return ring[:, s * BC:(s + 1) * BC]

        rbc_pool = ctx.enter_context(tc.tile_pool(name="rbc", bufs=2))

        ex_tiles = {}   # granule index -> ex tile (CAPB steps each)
        pend_R = None   # (broadcast reciprocal sbuf tile, application step)
        # Pool cannot read PSUM on TRN2, so both groups' multiplies run on
        # DVE (in-order, so g0/g1 need no semaphores between them); Pool
        # handles the renorm broadcast (SBUF-only) off the chain.
        mults = [nc.vector, nc.vector]

        # First chunk split small so step 0 starts ~9us earlier.
        chunk_steps = [CAPB, CH - CAPB] + [CH] * (T // CH - 1)
        chunk_t0 = np.cumsum([0] + chunk_steps[:-1]).tolist()
        for cs, ct0 in zip(chunk_steps, chunk_t0):
            xt = xin_pool.tile([L, cs * BC], F32, tag="xt")
            if ct0 == 0:
                # Finer DMA granularity so step 0 starts after ~1/4 of the
                # first block's x has landed (subtile deps gate per-slice).
                for q in range(4):
                    nc.sync.dma_start(
                        xt[:, q * 8 * BC:(q + 1) * 8 * BC],
                        xT[:, (ct0 + q * 8) * BC:(ct0 + (q + 1) * 8) * BC])
            else:
                nc.sync.dma_start(xt[:], xT[:, ct0 * BC:(ct0 + cs) * BC])
            for jj in range(cs // CAPB):
                j = (ct0 // CAPB) + jj      # capture block index
                jo = jj                      # granule offset within chunk
                ex = ex_pool.tile([L, CAPB * BC], DT)
                ngran = 4 if j == 0 else 1   # finer exp granules at startup
                for q in range(ngran):
                    gs = CAPB // ngran
                    sl = slice((jo * CAPB + q * gs) * BC,
                               (jo * CAPB + (q + 1) * gs) * BC)
                    esl = slice(q * gs * BC, (q + 1) * gs * BC)
                    nc.scalar.activation(
                        ex[:, esl], xt[:, sl],
                        mybir.ActivationFunctionType.Exp, bias=nbias[:],
                    )
                ex_tiles[j] = ex
                for i in range(CAPB):
                    t = j * CAPB + i
                    # Apply a pending renorm to this step's ex slice, each
                    # engine scaling the half it will consume (same-queue
                    # ordering, no extra semaphores on the chain).
                    if pend_R is not None and pend_R[1] == t:
                        R = pend_R[0]
                        for g in range(G):
                            sl = slice(i * BC + g * GS, i * BC + (g + 1) * GS)
                            mults[g].tensor_mul(ex[:, sl], ex[:, sl],
                                                R[:, g * GS:(g + 1) * GS])
                        pend_R = None
                    src = wslot(t)
                    dst = wslot(t + 1)
                    for g in range(G):
                        P = ps_pool.tile([L, GS], F32, tag=f"P{g}")
                        mm = nc.tensor.matmul(P[:], E_sb[:],
                                              src[:, g * GS:(g + 1) * GS],
                                              start=True, stop=True)
                        mm.ins.ldweights = False
                        mults[g].tensor_mul(dst[:, g * GS:(g + 1) * GS],
                                            ex[:, i * BC + g * GS:
                                               i * BC + (g + 1) * GS],
                                            P[:])
                    # Renorm trigger: tau = t = K(m+1); normalizer = the fp16
                    # state row 0 just written (any per-column scale works;
                    # the host uses the recorded fp16 reciprocal exactly).
                    # Broadcast it on Pool and fold into the ex slice of step
                    # tau+D, off the serial matmul/multiply chain.
                    if t % K == 0 and t > 0 and t + D <= T - 1:
                        m = t // K - 1
                        nc.vector.reciprocal(rh_sb[0:1, m * BC:(m + 1) * BC],
                                             dst[0:1, :])
                        Rbc = rbc_pool.tile([L, BC], DT)
                        nc.gpsimd.partition_broadcast(
                            Rbc[:], rh_sb[0:1, m * BC:(m + 1) * BC])
                        pend_R = (Rbc, t + D)
                # Capture row 127 of the finished ring (w_{32j+1..32j+32});
                # the double ring gives this DMA 32 steps of WAR slack.
                ring = rings[j % 2]
                nc.sync.dma_start(
                    hist[0:1, j * CAPB * BC:(j + 1) * CAPB * BC],
                    ring[127:128, :])
                if j - 2 in ex_tiles:
                    del ex_tiles[j - 2]

        # Final (E @ w_512)[127] for samples with len == T.
        Pf = ps_pool.tile([L, BC], F32, tag="P0")
        mmf = nc.tensor.matmul(Pf[:], E_sb[:], wslot(T), start=True, stop=True)
        mmf.ins.ldweights = False
        capf = state_pool.tile([L, BC], DT)
        nc.vector.tensor_copy(capf[:], Pf[:])
        nc.sync.dma_start(hist[0:1, T * BC:T * BC + BC], capf[127:128, :])
        nc.sync.dma_start(rhist[0:1, :], rh_sb[:])

    # Tile re-pairs every matmul with its own InstLdweights regardless of the
    # non-self-loading flag; strip those (keeping the single explicit load).
    # They carry no semaphore waits (verified: all waits live on the matmuls),
    # so deleting them does not disturb synchronization.
    removed = 0
    for fn in nc.m.functions:
        for bb in fn.blocks:
            insts = bb.instructions
            for i in range(len(insts) - 1, -1, -1):
                inst = insts[i]
                if (type(inst).__name__ == "InstLdweights"
                        and inst.name not in keep_ld_names):
                    si = inst.sync_info
                    assert si is None or (not si.on_wait and not si.on_update), \
                        f"ldweights {inst.name} carries sync; refusing to drop"
                    del insts[i]
                    removed += 1
    assert removed == 1025, f"expected 1025 redundant ldweights, got {removed}"

    nc.compile()
    return nc


def _get_nc():
    global _CACHED_NC
    if _CACHED_NC is None:
        _CACHED_NC = _build_bass()
    return _CACHED_NC


def run_on_device(x, transit_matrix, **spmd_kwargs):
    """Shard inputs, run the SPMD kernel on 8 cores, return per-core results."""
    xT = np.ascontiguousarray(np.asarray(x, np.float32).transpose(2, 1, 0))
    trT = np.ascontiguousarray(np.asarray(transit_matrix, np.float32).T)
    in_maps = []
    for c in range(NCORES):
        xc = np.ascontiguousarray(xT[:, :, c * BC:(c + 1) * BC]).reshape(L, T * BC)
        in_maps.append({"xT": xc, "trT": trT})
    nc = _get_nc()
    return bass_utils.run_bass_kernel_spmd(
        nc, in_maps, core_ids=list(range(NCORES)), **spmd_kwargs
    )


def finish_on_host(results, x, lengths):
    """Reconstruct alpha[b] in float64 from the device captures.

    fv_t = ln(w_t) + t*SCALE_LN + sum of ln(s_m) over norms applied before t
    (norm m: s_m = 1/r_m, r_m recorded; applied at step a_m = K(m+1)+D).
    For len < T the capture is w_{len+1}[127] = exp(x[b,len,127])/256 *
    (E @ w_len)[127] (with the step-len renorm folded in when a_m == len), which
    collapses to the uniform formula below; for len == T the tail capture is
    (E @ w_512)[127] directly.
    """
    lengths = np.asarray(lengths).astype(np.int64)
    x = np.asarray(x)
    alpha = np.empty(B, np.float64)
    for c in range(NCORES):
        hist = results[c]["hist"].reshape(-1).astype(np.float64)
        rh = results[c]["rhist"].reshape(-1).astype(np.float64)
        lnS = -np.log(rh.reshape(NNORM, BC))          # ln s_m per norm m
        cum = np.zeros((NNORM + 1, BC))
        cum[1:] = np.cumsum(lnS, axis=0)
        hist_blk = hist[:T * BC].reshape(T, BC)       # hist_blk[t-1] = w_t[127]
        cap512 = hist[T * BC:]

        ln = lengths[c * BC:(c + 1) * BC]             # (BC,)
        bi = np.arange(BC)
        full = ln == T
        nf = ~full
        out = np.empty(BC, np.float64)
        # Captures that underflowed deep into fp16 subnormals lose log
        # accuracy; flag them (NaN) for the exact host fallback in kernel().
        with np.errstate(divide="ignore", invalid="ignore"):
            out[full] = T * SCALE_LN + cum[NNORM, bi[full]] + np.log(
                np.where(cap512[full] < 3e-7, np.nan, cap512[full]))
        cap = hist_blk[ln[nf], bi[nf]]                # w_{len+1}[127]
        cap = np.where(cap < 3e-7, np.nan, cap)
        x127 = x[c * BC + bi[nf], ln[nf], 127].astype(np.float64)
        # norms applied at a_m = K(m+1)+D <= len: count = (len-D)//K, clipped
        nidx = np.clip((ln[nf] - D) // K, 0, NNORM)
        with np.errstate(divide="ignore", invalid="ignore"):
            out[nf] = (np.log(cap) - x127 + (ln[nf] + 1) * SCALE_LN
                       + cum[nidx, bi[nf]])
        alpha[c * BC:(c + 1) * BC] = out
    return alpha.astype(np.float32)


def _crf_alpha_single(xb, tr, length):
    """Exact single-sample CRF forward in float64 (rare-fallback path)."""
    NEG = -10000.0
    trd = np.asarray(tr, np.float64)
    fv = np.full(L, NEG)
    fv[0] = 0.0
    for t in range(int(length)):
        sc = trd + fv[None, :] + np.asarray(xb[t], np.float64)[:, None]
        m = sc.max(axis=1)
        fv = m + np.log(np.exp(sc - m[:, None]).sum(axis=1))
    term = fv + trd[L - 1]
    m = term.max()
    return m + np.log(np.exp(term - m).sum())


def kernel(x, transit_matrix, lengths):
    x = np.asarray(x, np.float32)
    assert x.shape == (B, T, L), x.shape
    res = run_on_device(x, transit_matrix)
    alpha = finish_on_host(res.results, x, lengths)
    # fp16 captures can in principle underflow to subnormal/zero for extreme
    # samples; recompute those few (if any) exactly on host.
    bad = ~np.isfinite(alpha)
    if bad.any():
        ln = np.asarray(lengths).astype(np.int64)
        for b in np.nonzero(bad)[0]:
            alpha[b] = _crf_alpha_single(x[b], transit_matrix, ln[b])
    return alpha


# revision 35
# speedup vs baseline: 1.0189x; 1.0132x over previous
"""CRF forward-algorithm kernel for Trainium2 (8 NeuronCores, data-parallel over batch).

Math: the reference computes, per sample b,
    fv_{t+1}[next] = x_t[next] + logsumexp_prev(transit[next, prev] + fv_t[prev])   (t < len_b)
    alpha[b] = logsumexp_i(fv_{len_b}[i] + transit[STOP, i])

In linear space with E = exp(transit) this is
    w_{t+1} = exp(x_t) * (E @ w_t),      fv_t = log(w_t) + c_t
so each timestep is one fp16 128x128 @ 128x32 matmul (PE) plus one elementwise
multiply.  The 512-step chain is serial, so wall time = 512 x round latency
(PE -> PSUM -> multiply -> SBUF -> PE).  The round is minimized by:
  * loading E into the PE array once (ldweights) and marking every step matmul
    non-self-loading, which removes the ~100ns weight reload that otherwise
    sits after the semaphore wait on the critical path;
  * splitting the 32 batch columns into two groups whose multiplies run
    concurrently on the Vector (DVE) and Pool engines, each reading its own
    PSUM bank, so per-round engine time is halved.
exp(x) is pre-scaled by 1/256 and the state is renormalized every K=8 steps:
the normalizer is the fp16 state row 0 (already in SBUF), its reciprocal is
recorded for the host, broadcast across partitions on Pool, and folded into
the exp(x) slice of step tau+D in each engine's own queue order, so the renorm
never adds semaphores to the serial chain.  Because alpha needs
(E @ w_len)[STOP] and STOP = 127 is the last row of E, the per-step capture is
just row 127 of the state; the state lives in two alternating 32-slot rings so
row 127 of a finished ring is DMA'd out with 32 steps of WAR slack.  The final
log/gather bookkeeping (O(B*T) scalar work) runs on host in float64 from the
captures.
"""

import sys

sys.path.insert(0, "/opt/trn_rl_repo")

import numpy as np
from contextlib import ExitStack

import concourse.bass as bass
import concourse.tile as tile
import concourse.mybir as mybir
from concourse import bacc, bass_utils



# Problem constants (hardcoded per contract).
B, T, L = 256, 512, 128
NCORES = 8
BC = B // NCORES          # 32 samples per core
K = 8                     # renormalization period
D = 4                     # renorm application delay (steps after tau)
CAPB = 32                 # capture block (ring size)
CH = 128                  # x chunk length in timesteps
NCAP = T // CAPB          # capture blocks
NNORM = 63                # norms m=0..62: tau=8(m+1)<=504, applied at tau+D
G = 2                     # batch groups: g0 -> Vector, g1 -> Pool
GS = BC // G
SCALE_LN = float(np.log(256.0))
F32 = mybir.dt.float32
DT = mybir.dt.float16     # state/weights dtype

_CACHED_NC = None





def _build_bass():
    """Build the single-core Bass program (shared SPMD across 8 cores)."""
    nc = bacc.Bacc("TRN2", debug=False)

    xT = nc.dram_tensor("xT", [L, T * BC], F32, kind="ExternalInput").ap()
    trT = nc.dram_tensor("trT", [L, L], F32, kind="ExternalInput").ap()
    # hist[j*CAPB*BC + s*BC + b] = w_{32j+1+s}[127, b]; tail BC entries are
    # (E @ w_512)[127].
    hist = nc.dram_tensor("hist", [1, T * BC + BC], DT, kind="ExternalOutput").ap()
    rhist = nc.dram_tensor("rhist", [1, NNORM * BC], DT, kind="ExternalOutput").ap()

    keep_ld_names = set()
    with tile.TileContext(nc) as tc, ExitStack() as ctx, \
            nc.allow_low_precision(reason="fp16 state validated against f64 ref"):
        # One static pool for constants/state/renorm tiles (fewer pools ->
        # shorter serial event-semaphore teardown at program end).
        const_pool = ctx.enter_context(tc.tile_pool(name="const", bufs=1))
        state_pool = const_pool
        rbc_pool = const_pool
        xin_pool = ctx.enter_context(tc.tile_pool(name="xin", bufs=2))
        ex_pool = ctx.enter_context(tc.tile_pool(name="ex", bufs=3))
        ps_pool = ctx.enter_context(tc.tile_pool(name="ps", bufs=4, space="PSUM"))

        # Constants.
        nbias = const_pool.tile([L, 1], F32)
        nc.vector.memset(nbias[:], -SCALE_LN)

        tr_sb = const_pool.tile([L, L], F32)
        nc.sync.dma_start(tr_sb[:], trT[:, :])
        E_sb = const_pool.tile([L, L], DT)
        nc.scalar.activation(E_sb[:], tr_sb[:], mybir.ActivationFunctionType.Exp)
        # E is loaded into the PE array exactly once and stays resident for
        # the whole chain: every matmul below is flagged non-self-loading and
        # the redundant per-matmul InstLdweights that Tile re-inserts are
        # deleted from the module after the TileContext exits (they carry no
        # semaphore waits, so removal is sync-safe).  This takes the ~100ns
        # 128-row weight reload off the serial matmul->multiply chain.
        lw = nc.tensor.ldweights(E_sb[:])
        keep_ld_names.add(lw.ins.name)

        # Reciprocal history (one fp16 reciprocal per norm per sample).
        rh_sb = state_pool.tile([1, NNORM * BC], DT)

        # Two state rings: ring(j) = j%2 holds w_{32j+1..32j+32} in slots 0..31.
        WA = state_pool.tile([L, CAPB * BC], DT)
        WB = state_pool.tile([L, CAPB * BC], DT)
        rings = [WA, WB]
        # w_0 = onehot(START=0) lives at ring 1, slot 31.
        nc.vector.memset(WB[:, (CAPB - 1) * BC:CAPB * BC], 0.0)
        nc.vector.memset(WB[0:1, (CAPB - 1) * BC:CAPB * BC], 1.0)

        def wslot(t):
            """AP of w_t (full BC columns)."""
            ring = rings[((t - 1) // CAPB) % 2]
            s = (t - 1) % CAPB
            return ring[:, s * BC:(s + 1) * BC]

        ex_tiles = {}   # granule index -> ex tile (CAPB steps each)
        pend_R = None   # (broadcast reciprocal sbuf tile, application step)
        pend_rcp = None  # deferred second half of the renorm reciprocal
        # Pool cannot read PSUM on TRN2, so both groups' multiplies run on
        # DVE (in-order, so g0/g1 need no semaphores between them); Pool
        # handles the renorm broadcast (SBUF-only) off the chain.
        mults = [nc.vector, nc.vector]

        # First chunk split small so step 0 starts ~9us earlier.
        chunk_steps = [CAPB, CH - CAPB] + [CH] * (T // CH - 1)
        chunk_t0 = np.cumsum([0] + chunk_steps[:-1]).tolist()
        for cs, ct0 in zip(chunk_steps, chunk_t0):
            xt = xin_pool.tile([L, cs * BC], F32, tag="xt")
            if ct0 == 0:
                # Finer DMA granularity so step 0 starts after just the first
                # few steps of x have landed (subtile deps gate per-slice).
                for q0, qn in ((0, 4), (4, 4), (8, 8), (16, 16)):
                    nc.sync.dma_start(
                        xt[:, q0 * BC:(q0 + qn) * BC],
                        xT[:, q0 * BC:(q0 + qn) * BC])
            else:
                nc.sync.dma_start(xt[:], xT[:, ct0 * BC:(ct0 + cs) * BC])
            for jj in range(cs // CAPB):
                j = (ct0 // CAPB) + jj      # capture block index
                jo = jj                      # granule offset within chunk
                ex = ex_pool.tile([L, CAPB * BC], DT)
                # Finer exp granules at startup so step 0 begins after ~4
                # steps of x instead of a full 32-step block.
                grans = ((0, 4), (4, 4), (8, 8), (16, 16)) if j == 0 \
                    else ((0, CAPB),)
                for q0, qn in grans:
                    sl = slice((jo * CAPB + q0) * BC,
                               (jo * CAPB + q0 + qn) * BC)
                    esl = slice(q0 * BC, (q0 + qn) * BC)
                    nc.scalar.activation(
                        ex[:, esl], xt[:, sl],
                        mybir.ActivationFunctionType.Exp, bias=nbias[:],
                    )
                ex_tiles[j] = ex
                for i in range(CAPB):
                    t = j * CAPB + i
                    # Apply a pending renorm to this step's ex slice (one op;
                    # same-queue ordering keeps it off the serial chain).
                    if pend_R is not None and pend_R[1] == t:
                        R = pend_R[0]
                        sl = slice(i * BC, (i + 1) * BC)
                        nc.vector.tensor_mul(ex[:, sl], ex[:, sl], R[:])
                        pend_R = None
                    # Second half of the renorm reciprocal, one step after the
                    # first ([1,16] pieces fit the per-step DVE slack), then
                    # the Pool broadcast of the completed [1,32] row.
                    if pend_rcp is not None:
                        out_ap, in_ap, m_, app_t = pend_rcp
                        nc.vector.reciprocal(out_ap, in_ap)
                        Rbc = rbc_pool.tile([L, BC], DT, tag="rbc", bufs=2)
                        nc.gpsimd.partition_broadcast(
                            Rbc[:], rh_sb[0:1, m_ * BC:(m_ + 1) * BC])
                        pend_R = (Rbc, app_t)
                        pend_rcp = None
                    src = wslot(t)
                    dst = wslot(t + 1)
                    for g in range(G):
                        P = ps_pool.tile([L, GS], F32, tag=f"P{g}")
                        mm = nc.tensor.matmul(P[:], E_sb[:],
                                              src[:, g * GS:(g + 1) * GS],
                                              start=True, stop=True)
                        mm.ins.ldweights = False
                        mults[g].tensor_mul(dst[:, g * GS:(g + 1) * GS],
                                            ex[:, i * BC + g * GS:
                                               i * BC + (g + 1) * GS],
                                            P[:])
                    # Renorm trigger: tau = t = K(m+1); normalizer = the fp16
                    # state row 0 just written (any per-column scale works;
                    # the host uses the recorded fp16 reciprocal exactly).
                    # Broadcast it on Pool and fold into the ex slice of step
                    # tau+D, off the serial matmul/multiply chain.
                    if t % K == 0 and t > 0 and t + D <= T - 1:
                        m = t // K - 1
                        nc.vector.reciprocal(rh_sb[0:1, m * BC:m * BC + GS],
                                             dst[0:1, 0:GS])
                        pend_rcp = (rh_sb[0:1, m * BC + GS:(m + 1) * BC],
                                    dst[0:1, GS:BC], m, t + D)
                # Capture row 127 of the finished ring (w_{32j+1..32j+32});
                # the double ring gives this DMA 32 steps of WAR slack.
                ring = rings[j % 2]
                nc.sync.dma_start(
                    hist[0:1, j * CAPB * BC:(j + 1) * CAPB * BC],
                    ring[127:128, :])
                if j - 2 in ex_tiles:
                    del ex_tiles[j - 2]

        # Final (E @ w_512)[127] for samples with len == T.
        Pf = ps_pool.tile([L, BC], F32, tag="P0")
        mmf = nc.tensor.matmul(Pf[:], E_sb[:], wslot(T), start=True, stop=True)
        mmf.ins.ldweights = False
        capf = state_pool.tile([L, BC], DT)
        nc.vector.tensor_copy(capf[:], Pf[:])
        nc.sync.dma_start(hist[0:1, T * BC:T * BC + BC], capf[127:128, :])
        nc.sync.dma_start(rhist[0:1, :], rh_sb[:])

    # Tile re-pairs every matmul with its own InstLdweights regardless of the
    # non-self-loading flag; strip those (keeping the single explicit load).
    # They carry no semaphore waits (verified: all waits live on the matmuls),
    # so deleting them does not disturb synchronization.
    removed = 0
    for fn in nc.m.functions:
        for bb in fn.blocks:
            insts = bb.instructions
            for i in range(len(insts) - 1, -1, -1):
                inst = insts[i]
                if (type(inst).__name__ == "InstLdweights"
                        and inst.name not in keep_ld_names):
                    si = inst.sync_info
                    assert si is None or (not si.on_wait and not si.on_update), \
                        f"ldweights {inst.name} carries sync; refusing to drop"
                    del insts[i]
                    removed += 1
    assert removed == 1025, f"expected 1025 redundant ldweights, got {removed}"

    nc.compile()
    return nc


def _get_nc():
    global _CACHED_NC
    if _CACHED_NC is None:
        _CACHED_NC = _build_bass()
    return _CACHED_NC


def run_on_device(x, transit_matrix, **spmd_kwargs):
    """Shard inputs, run the SPMD kernel on 8 cores, return per-core results."""
    xT = np.ascontiguousarray(np.asarray(x, np.float32).transpose(2, 1, 0))
    trT = np.ascontiguousarray(np.asarray(transit_matrix, np.float32).T)
    in_maps = []
    for c in range(NCORES):
        xc = np.ascontiguousarray(xT[:, :, c * BC:(c + 1) * BC]).reshape(L, T * BC)
        in_maps.append({"xT": xc, "trT": trT})
    nc = _get_nc()
    return bass_utils.run_bass_kernel_spmd(
        nc, in_maps, core_ids=list(range(NCORES)), **spmd_kwargs
    )


def finish_on_host(results, x, lengths):
    """Reconstruct alpha[b] in float64 from the device captures.

    fv_t = ln(w_t) + t*SCALE_LN + sum of ln(s_m) over norms applied before t
    (norm m: s_m = 1/r_m, r_m recorded; applied at step a_m = K(m+1)+D).
    For len < T the capture is w_{len+1}[127] = exp(x[b,len,127])/256 *
    (E @ w_len)[127] (with the step-len renorm folded in when a_m == len), which
    collapses to the uniform formula below; for len == T the tail capture is
    (E @ w_512)[127] directly.
    """
    lengths = np.asarray(lengths).astype(np.int64)
    x = np.asarray(x)
    alpha = np.empty(B, np.float64)
    for c in range(NCORES):
        hist = results[c]["hist"].reshape(-1).astype(np.float64)
        rh = results[c]["rhist"].reshape(-1).astype(np.float64)
        lnS = -np.log(rh.reshape(NNORM, BC))          # ln s_m per norm m
        cum = np.zeros((NNORM + 1, BC))
        cum[1:] = np.cumsum(lnS, axis=0)
        hist_blk = hist[:T * BC].reshape(T, BC)       # hist_blk[t-1] = w_t[127]
        cap512 = hist[T * BC:]

        ln = lengths[c * BC:(c + 1) * BC]             # (BC,)
        bi = np.arange(BC)
        full = ln == T
        nf = ~full
        out = np.empty(BC, np.float64)
        # Captures that underflowed deep into fp16 subnormals lose log
        # accuracy; flag them (NaN) for the exact host fallback in kernel().
        with np.errstate(divide="ignore", invalid="ignore"):
            out[full] = T * SCALE_LN + cum[NNORM, bi[full]] + np.log(
                np.where(cap512[full] < 3e-7, np.nan, cap512[full]))
        cap = hist_blk[ln[nf], bi[nf]]                # w_{len+1}[127]
        cap = np.where(cap < 3e-7, np.nan, cap)
        x127 = x[c * BC + bi[nf], ln[nf], 127].astype(np.float64)
        # norms applied at a_m = K(m+1)+D <= len: count = (len-D)//K, clipped
        nidx = np.clip((ln[nf] - D) // K, 0, NNORM)
        with np.errstate(divide="ignore", invalid="ignore"):
            out[nf] = (np.log(cap) - x127 + (ln[nf] + 1) * SCALE_LN
                       + cum[nidx, bi[nf]])
        alpha[c * BC:(c + 1) * BC] = out
    return alpha.astype(np.float32)


def _crf_alpha_single(xb, tr, length):
    """Exact single-sample CRF forward in float64 (rare-fallback path)."""
    NEG = -10000.0
    trd = np.asarray(tr, np.float64)
    fv = np.full(L, NEG)
    fv[0] = 0.0
    for t in range(int(length)):
        sc = trd + fv[None, :] + np.asarray(xb[t], np.float64)[:, None]
        m = sc.max(axis=1)
        fv = m + np.log(np.exp(sc - m[:, None]).sum(axis=1))
    term = fv + trd[L - 1]
    m = term.max()
    return m + np.log(np.exp(term - m).sum())


def kernel(x, transit_matrix, lengths):
    x = np.asarray(x, np.float32)
    assert x.shape == (B, T, L), x.shape
    res = run_on_device(x, transit_matrix)
    alpha = finish_on_host(res.results, x, lengths)
    # fp16 captures can in principle underflow to subnormal/zero for extreme
    # samples; recompute those few (if any) exactly on host.
    bad = ~np.isfinite(alpha)
    if bad.any():
        ln = np.asarray(lengths).astype(np.int64)
        for b in np.nonzero(bad)[0]:
            alpha[b] = _crf_alpha_single(x[b], transit_matrix, ln[b])
    return alpha


# revision 37
# speedup vs baseline: 1.0234x; 1.0044x over previous
"""CRF forward-algorithm kernel for Trainium2 (8 NeuronCores, data-parallel over batch).

Math: the reference computes, per sample b,
    fv_{t+1}[next] = x_t[next] + logsumexp_prev(transit[next, prev] + fv_t[prev])   (t < len_b)
    alpha[b] = logsumexp_i(fv_{len_b}[i] + transit[STOP, i])

In linear space with E = exp(transit) this is
    w_{t+1} = exp(x_t) * (E @ w_t),      fv_t = log(w_t) + c_t
so each timestep is one fp16 128x128 @ 128x32 matmul (PE) plus one elementwise
multiply.  The 512-step chain is serial, so wall time = 512 x round latency
(PE -> PSUM -> multiply -> SBUF -> PE).  The round is minimized by:
  * loading E into the PE array once (ldweights) and marking every step matmul
    non-self-loading, which removes the ~100ns weight reload that otherwise
    sits after the semaphore wait on the critical path;
  * splitting the 32 batch columns into two groups whose multiplies run
    concurrently on the Vector (DVE) and Pool engines, each reading its own
    PSUM bank, so per-round engine time is halved.
exp(x) is pre-scaled by 1/256 and the state is renormalized every K=8 steps:
the normalizer is the fp16 state row 0 (already in SBUF), its reciprocal is
recorded for the host, broadcast across partitions on Pool, and folded into
the exp(x) slice of step tau+D in each engine's own queue order, so the renorm
never adds semaphores to the serial chain.  Because alpha needs
(E @ w_len)[STOP] and STOP = 127 is the last row of E, the per-step capture is
just row 127 of the state; the state lives in two alternating 32-slot rings so
row 127 of a finished ring is DMA'd out with 32 steps of WAR slack.  The final
log/gather bookkeeping (O(B*T) scalar work) runs on host in float64 from the
captures.
"""

import sys

sys.path.insert(0, "/opt/trn_rl_repo")

import numpy as np
from contextlib import ExitStack

import concourse.bass as bass
import concourse.tile as tile
import concourse.mybir as mybir
from concourse import bacc, bass_utils



# Problem constants (hardcoded per contract).
B, T, L = 256, 512, 128
NCORES = 8
BC = B // NCORES          # 32 samples per core
K = 8                     # renormalization period
D = 4                     # renorm application delay (steps after tau)
CAPB = 32                 # capture block (ring size)
CH = 128                  # x chunk length in timesteps
NCAP = T // CAPB          # capture blocks
NNORM = 63                # norms m=0..62: tau=8(m+1)<=504, applied at tau+D
G = 2                     # batch groups: g0 -> Vector, g1 -> Pool
GS = BC // G
SCALE_LN = float(np.log(256.0))
F32 = mybir.dt.float32
DT = mybir.dt.float16     # state/weights dtype

_CACHED_NC = None





def _build_bass():
    """Build the single-core Bass program (shared SPMD across 8 cores)."""
    nc = bacc.Bacc("TRN2", debug=False)

    xT = nc.dram_tensor("xT", [L, T * BC], F32, kind="ExternalInput").ap()
    trT = nc.dram_tensor("trT", [L, L], F32, kind="ExternalInput").ap()
    # hist[j*CAPB*BC + s*BC + b] = w_{32j+1+s}[127, b]; tail BC entries are
    # (E @ w_512)[127].
    hist = nc.dram_tensor("hist", [1, T * BC + BC], DT, kind="ExternalOutput").ap()
    rhist = nc.dram_tensor("rhist", [1, NNORM * BC], DT, kind="ExternalOutput").ap()

    keep_ld_names = set()
    with tile.TileContext(nc) as tc, ExitStack() as ctx, \
            nc.allow_low_precision(reason="fp16 state validated against f64 ref"):
        # One static pool for constants/state/renorm tiles (fewer pools ->
        # shorter serial event-semaphore teardown at program end).
        const_pool = ctx.enter_context(tc.tile_pool(name="const", bufs=1))
        state_pool = const_pool
        rbc_pool = const_pool
        xin_pool = ctx.enter_context(tc.tile_pool(name="xin", bufs=2))
        ex_pool = ctx.enter_context(tc.tile_pool(name="ex", bufs=3))
        ps_pool = ctx.enter_context(tc.tile_pool(name="ps", bufs=4, space="PSUM"))

        # Constants.
        nbias = const_pool.tile([L, 1], F32)
        nc.vector.memset(nbias[:], -SCALE_LN)

        tr_sb = const_pool.tile([L, L], F32)
        nc.sync.dma_start(tr_sb[:], trT[:, :])
        E_sb = const_pool.tile([L, L], DT)
        nc.scalar.activation(E_sb[:], tr_sb[:], mybir.ActivationFunctionType.Exp)
        # E is loaded into the PE array exactly once and stays resident for
        # the whole chain: every matmul below is flagged non-self-loading and
        # the redundant per-matmul InstLdweights that Tile re-inserts are
        # deleted from the module after the TileContext exits (they carry no
        # semaphore waits, so removal is sync-safe).  This takes the ~100ns
        # 128-row weight reload off the serial matmul->multiply chain.
        lw = nc.tensor.ldweights(E_sb[:])
        keep_ld_names.add(lw.ins.name)

        # Reciprocal history (one fp16 reciprocal per norm per sample).
        rh_sb = state_pool.tile([1, NNORM * BC], DT)

        # Two broadcast-reciprocal buffers, alternated per renorm (static
        # tiles, not a rotating pool: each pool-tile allocation leaves a
        # per-queue release semaphore that serializes at program end).
        RbcA = const_pool.tile([L, BC], DT)
        RbcB = const_pool.tile([L, BC], DT)
        Rbcs = [RbcA, RbcB]

        # Two state rings: ring(j) = j%2 holds w_{32j+1..32j+32} in slots 0..31.
        WA = state_pool.tile([L, CAPB * BC], DT)
        WB = state_pool.tile([L, CAPB * BC], DT)
        rings = [WA, WB]
        # w_0 = onehot(START=0) lives at ring 1, slot 31.
        nc.vector.memset(WB[:, (CAPB - 1) * BC:CAPB * BC], 0.0)
        nc.vector.memset(WB[0:1, (CAPB - 1) * BC:CAPB * BC], 1.0)

        def wslot(t):
            """AP of w_t (full BC columns)."""
            ring = rings[((t - 1) // CAPB) % 2]
            s = (t - 1) % CAPB
            return ring[:, s * BC:(s + 1) * BC]

        ex_tiles = {}   # granule index -> ex tile (CAPB steps each)
        pend_R = None   # (broadcast reciprocal sbuf tile, application step)
        pend_rcp = None  # deferred second half of the renorm reciprocal
        # Pool cannot read PSUM on TRN2, so both groups' multiplies run on
        # DVE (in-order, so g0/g1 need no semaphores between them); Pool
        # handles the renorm broadcast (SBUF-only) off the chain.
        mults = [nc.vector, nc.vector]

        # First chunk split small so step 0 starts ~9us earlier.
        chunk_steps = [CAPB, CH - CAPB] + [CH] * (T // CH - 1)
        chunk_t0 = np.cumsum([0] + chunk_steps[:-1]).tolist()
        for cs, ct0 in zip(chunk_steps, chunk_t0):
            xt = xin_pool.tile([L, cs * BC], F32, tag="xt")
            if ct0 == 0:
                # Finer DMA granularity so step 0 starts after just the first
                # few steps of x have landed (subtile deps gate per-slice).
                for q0, qn in ((0, 4), (4, 4), (8, 8), (16, 16)):
                    nc.sync.dma_start(
                        xt[:, q0 * BC:(q0 + qn) * BC],
                        xT[:, q0 * BC:(q0 + qn) * BC])
            else:
                nc.sync.dma_start(xt[:], xT[:, ct0 * BC:(ct0 + cs) * BC])
            for jj in range(cs // CAPB):
                j = (ct0 // CAPB) + jj      # capture block index
                jo = jj                      # granule offset within chunk
                ex = ex_pool.tile([L, CAPB * BC], DT)
                # Finer exp granules at startup so step 0 begins after ~4
                # steps of x instead of a full 32-step block.
                grans = ((0, 4), (4, 4), (8, 8), (16, 16)) if j == 0 \
                    else ((0, CAPB),)
                for q0, qn in grans:
                    sl = slice((jo * CAPB + q0) * BC,
                               (jo * CAPB + q0 + qn) * BC)
                    esl = slice(q0 * BC, (q0 + qn) * BC)
                    nc.scalar.activation(
                        ex[:, esl], xt[:, sl],
                        mybir.ActivationFunctionType.Exp, bias=nbias[:],
                    )
                ex_tiles[j] = ex
                for i in range(CAPB):
                    t = j * CAPB + i
                    # Apply a pending renorm to this step's ex slice (one op;
                    # same-queue ordering keeps it off the serial chain).
                    if pend_R is not None and pend_R[1] == t:
                        R = pend_R[0]
                        sl = slice(i * BC, (i + 1) * BC)
                        nc.vector.tensor_mul(ex[:, sl], ex[:, sl], R[:])
                        pend_R = None
                    # Second half of the renorm reciprocal, one step after the
                    # first ([1,16] pieces fit the per-step DVE slack), then
                    # the Pool broadcast of the completed [1,32] row.
                    if pend_rcp is not None:
                        out_ap, in_ap, m_, app_t = pend_rcp
                        nc.vector.reciprocal(out_ap, in_ap)
                        Rbc = Rbcs[m_ % 2]
                        nc.gpsimd.partition_broadcast(
                            Rbc[:], rh_sb[0:1, m_ * BC:(m_ + 1) * BC])
                        pend_R = (Rbc, app_t)
                        pend_rcp = None
                    src = wslot(t)
                    dst = wslot(t + 1)
                    for g in range(G):
                        P = ps_pool.tile([L, GS], F32, tag=f"P{g}")
                        mm = nc.tensor.matmul(P[:], E_sb[:],
                                              src[:, g * GS:(g + 1) * GS],
                                              start=True, stop=True)
                        mm.ins.ldweights = False
                        mults[g].tensor_mul(dst[:, g * GS:(g + 1) * GS],
                                            ex[:, i * BC + g * GS:
                                               i * BC + (g + 1) * GS],
                                            P[:])
                    # Renorm trigger: tau = t = K(m+1); normalizer = the fp16
                    # state row 0 just written (any per-column scale works;
                    # the host uses the recorded fp16 reciprocal exactly).
                    # Broadcast it on Pool and fold into the ex slice of step
                    # tau+D, off the serial matmul/multiply chain.
                    if t % K == 0 and t > 0 and t + D <= T - 1:
                        m = t // K - 1
                        nc.vector.reciprocal(rh_sb[0:1, m * BC:m * BC + GS],
                                             dst[0:1, 0:GS])
                        pend_rcp = (rh_sb[0:1, m * BC + GS:(m + 1) * BC],
                                    dst[0:1, GS:BC], m, t + D)
                # Capture row 127 of the finished ring (w_{32j+1..32j+32});
                # the double ring gives this DMA 32 steps of WAR slack.
                ring = rings[j % 2]
                nc.sync.dma_start(
                    hist[0:1, j * CAPB * BC:(j + 1) * CAPB * BC],
                    ring[127:128, :])
                if j - 2 in ex_tiles:
                    del ex_tiles[j - 2]

        # Final (E @ w_512)[127] for samples with len == T.
        Pf = ps_pool.tile([L, BC], F32, tag="P0")
        mmf = nc.tensor.matmul(Pf[:], E_sb[:], wslot(T), start=True, stop=True)
        mmf.ins.ldweights = False
        capf = state_pool.tile([L, BC], DT)
        nc.vector.tensor_copy(capf[:], Pf[:])
        nc.sync.dma_start(hist[0:1, T * BC:T * BC + BC], capf[127:128, :])
        nc.sync.dma_start(rhist[0:1, :], rh_sb[:])

    # Tile re-pairs every matmul with its own InstLdweights regardless of the
    # non-self-loading flag; strip those (keeping the single explicit load).
    # They carry no semaphore waits (verified: all waits live on the matmuls),
    # so deleting them does not disturb synchronization.
    removed = 0
    for fn in nc.m.functions:
        for bb in fn.blocks:
            insts = bb.instructions
            for i in range(len(insts) - 1, -1, -1):
                inst = insts[i]
                if (type(inst).__name__ == "InstLdweights"
                        and inst.name not in keep_ld_names):
                    si = inst.sync_info
                    assert si is None or (not si.on_wait and not si.on_update), \
                        f"ldweights {inst.name} carries sync; refusing to drop"
                    del insts[i]
                    removed += 1
    assert removed == 1025, f"expected 1025 redundant ldweights, got {removed}"

    nc.compile()
    return nc


def _get_nc():
    global _CACHED_NC
    if _CACHED_NC is None:
        _CACHED_NC = _build_bass()
    return _CACHED_NC


def run_on_device(x, transit_matrix, **spmd_kwargs):
    """Shard inputs, run the SPMD kernel on 8 cores, return per-core results."""
    xT = np.ascontiguousarray(np.asarray(x, np.float32).transpose(2, 1, 0))
    trT = np.ascontiguousarray(np.asarray(transit_matrix, np.float32).T)
    in_maps = []
    for c in range(NCORES):
        xc = np.ascontiguousarray(xT[:, :, c * BC:(c + 1) * BC]).reshape(L, T * BC)
        in_maps.append({"xT": xc, "trT": trT})
    nc = _get_nc()
    return bass_utils.run_bass_kernel_spmd(
        nc, in_maps, core_ids=list(range(NCORES)), **spmd_kwargs
    )


def finish_on_host(results, x, lengths):
    """Reconstruct alpha[b] in float64 from the device captures.

    fv_t = ln(w_t) + t*SCALE_LN + sum of ln(s_m) over norms applied before t
    (norm m: s_m = 1/r_m, r_m recorded; applied at step a_m = K(m+1)+D).
    For len < T the capture is w_{len+1}[127] = exp(x[b,len,127])/256 *
    (E @ w_len)[127] (with the step-len renorm folded in when a_m == len), which
    collapses to the uniform formula below; for len == T the tail capture is
    (E @ w_512)[127] directly.
    """
    lengths = np.asarray(lengths).astype(np.int64)
    x = np.asarray(x)
    alpha = np.empty(B, np.float64)
    for c in range(NCORES):
        hist = results[c]["hist"].reshape(-1).astype(np.float64)
        rh = results[c]["rhist"].reshape(-1).astype(np.float64)
        lnS = -np.log(rh.reshape(NNORM, BC))          # ln s_m per norm m
        cum = np.zeros((NNORM + 1, BC))
        cum[1:] = np.cumsum(lnS, axis=0)
        hist_blk = hist[:T * BC].reshape(T, BC)       # hist_blk[t-1] = w_t[127]
        cap512 = hist[T * BC:]

        ln = lengths[c * BC:(c + 1) * BC]             # (BC,)
        bi = np.arange(BC)
        full = ln == T
        nf = ~full
        out = np.empty(BC, np.float64)
        # Captures that underflowed deep into fp16 subnormals lose log
        # accuracy; flag them (NaN) for the exact host fallback in kernel().
        with np.errstate(divide="ignore", invalid="ignore"):
            out[full] = T * SCALE_LN + cum[NNORM, bi[full]] + np.log(
                np.where(cap512[full] < 3e-7, np.nan, cap512[full]))
        cap = hist_blk[ln[nf], bi[nf]]                # w_{len+1}[127]
        cap = np.where(cap < 3e-7, np.nan, cap)
        x127 = x[c * BC + bi[nf], ln[nf], 127].astype(np.float64)
        # norms applied at a_m = K(m+1)+D <= len: count = (len-D)//K, clipped
        nidx = np.clip((ln[nf] - D) // K, 0, NNORM)
        with np.errstate(divide="ignore", invalid="ignore"):
            out[nf] = (np.log(cap) - x127 + (ln[nf] + 1) * SCALE_LN
                       + cum[nidx, bi[nf]])
        alpha[c * BC:(c + 1) * BC] = out
    return alpha.astype(np.float32)


def _crf_alpha_single(xb, tr, length):
    """Exact single-sample CRF forward in float64 (rare-fallback path)."""
    NEG = -10000.0
    trd = np.asarray(tr, np.float64)
    fv = np.full(L, NEG)
    fv[0] = 0.0
    for t in range(int(length)):
        sc = trd + fv[None, :] + np.asarray(xb[t], np.float64)[:, None]
        m = sc.max(axis=1)
        fv = m + np.log(np.exp(sc - m[:, None]).sum(axis=1))
    term = fv + trd[L - 1]
    m = term.max()
    return m + np.log(np.exp(term - m).sum())


def kernel(x, transit_matrix, lengths):
    x = np.asarray(x, np.float32)
    assert x.shape == (B, T, L), x.shape
    res = run_on_device(x, transit_matrix)
    alpha = finish_on_host(res.results, x, lengths)
    # fp16 captures can in principle underflow to subnormal/zero for extreme
    # samples; recompute those few (if any) exactly on host.
    bad = ~np.isfinite(alpha)
    if bad.any():
        ln = np.asarray(lengths).astype(np.int64)
        for b in np.nonzero(bad)[0]:
            alpha[b] = _crf_alpha_single(x[b], transit_matrix, ln[b])
    return alpha


# revision 41
# speedup vs baseline: 1.0433x; 1.0194x over previous
"""CRF forward-algorithm kernel for Trainium2 (8 NeuronCores, data-parallel over batch).

Math: the reference computes, per sample b,
    fv_{t+1}[next] = x_t[next] + logsumexp_prev(transit[next, prev] + fv_t[prev])   (t < len_b)
    alpha[b] = logsumexp_i(fv_{len_b}[i] + transit[STOP, i])

In linear space with E = exp(transit) this is
    w_{t+1} = exp(x_t) * (E @ w_t),      fv_t = log(w_t) + c_t
so each timestep is one fp16 128x128 @ 128x32 matmul (PE) plus one elementwise
multiply.  The 512-step chain is serial, so wall time = 512 x round latency
(PE -> PSUM -> multiply -> SBUF -> PE).  The round is minimized by:
  * loading E into the PE array once (ldweights) and marking every step matmul
    non-self-loading, which removes the ~100ns weight reload that otherwise
    sits after the semaphore wait on the critical path;
  * splitting the 32 batch columns into two groups whose multiplies run
    concurrently on the Vector (DVE) and Pool engines, each reading its own
    PSUM bank, so per-round engine time is halved.
exp(x) is pre-scaled by 1/256 and the state is renormalized every K=8 steps:
the normalizer is the fp16 state row 0 (already in SBUF), its reciprocal is
recorded for the host, broadcast across partitions on Pool, and folded into
the exp(x) slice of step tau+D in each engine's own queue order, so the renorm
never adds semaphores to the serial chain.  Because alpha needs
(E @ w_len)[STOP] and STOP = 127 is the last row of E, the per-step capture is
just row 127 of the state; the state lives in two alternating 32-slot rings so
row 127 of a finished ring is DMA'd out with 32 steps of WAR slack.  The final
log/gather bookkeeping (O(B*T) scalar work) runs on host in float64 from the
captures.
"""

import sys

sys.path.insert(0, "/opt/trn_rl_repo")

import numpy as np
from contextlib import ExitStack

import concourse.bass as bass
import concourse.tile as tile
import concourse.mybir as mybir
from concourse import bacc, bass_utils



# Problem constants (hardcoded per contract).
B, T, L = 256, 512, 128
NCORES = 8
BC = B // NCORES          # 32 samples per core
K = 8                     # renormalization period
D = 4                     # renorm application delay (steps after tau)
CAPB = 32                 # capture block (ring size)
CH = 128                  # x chunk length in timesteps
NCAP = T // CAPB          # capture blocks
NNORM = 63                # norms m=0..62: tau=8(m+1)<=504, applied at tau+D
G = 3                     # batch groups (all multiplies on DVE)
GB = [0, 11, 22, 32]      # group column boundaries
GS = BC // 2              # renorm reciprocal half width
SCALE_LN = float(np.log(256.0))
F32 = mybir.dt.float32
DT = mybir.dt.float16     # state/weights dtype

_CACHED_NC = None





def _build_bass():
    """Build the single-core Bass program (shared SPMD across 8 cores)."""
    nc = bacc.Bacc("TRN2", debug=False)

    xT = nc.dram_tensor("xT", [L, T * BC], F32, kind="ExternalInput").ap()
    trT = nc.dram_tensor("trT", [L, L], F32, kind="ExternalInput").ap()
    # hist[j*CAPB*BC + s*BC + b] = w_{32j+1+s}[127, b]; tail BC entries are
    # (E @ w_512)[127].
    hist = nc.dram_tensor("hist", [1, T * BC + BC], DT, kind="ExternalOutput").ap()
    rhist = nc.dram_tensor("rhist", [1, NNORM * BC], DT, kind="ExternalOutput").ap()

    keep_ld_names = set()
    with tile.TileContext(nc) as tc, ExitStack() as ctx, \
            nc.allow_low_precision(reason="fp16 state validated against f64 ref"):
        # One static pool for constants/state/renorm tiles (fewer pools ->
        # shorter serial event-semaphore teardown at program end).
        const_pool = ctx.enter_context(tc.tile_pool(name="const", bufs=1))
        state_pool = const_pool
        rbc_pool = const_pool
        xin_pool = ctx.enter_context(tc.tile_pool(name="xin", bufs=2))
        ex_pool = ctx.enter_context(tc.tile_pool(name="ex", bufs=3))
        ps_pool = ctx.enter_context(tc.tile_pool(name="ps", bufs=2, space="PSUM"))

        # Constants.
        nbias = const_pool.tile([L, 1], F32)
        nc.vector.memset(nbias[:], -SCALE_LN)

        tr_sb = const_pool.tile([L, L], F32)
        nc.sync.dma_start(tr_sb[:], trT[:, :])
        E_sb = const_pool.tile([L, L], DT)
        nc.scalar.activation(E_sb[:], tr_sb[:], mybir.ActivationFunctionType.Exp)
        # E is loaded into the PE array exactly once and stays resident for
        # the whole chain: every matmul below is flagged non-self-loading and
        # the redundant per-matmul InstLdweights that Tile re-inserts are
        # deleted from the module after the TileContext exits (they carry no
        # semaphore waits, so removal is sync-safe).  This takes the ~100ns
        # 128-row weight reload off the serial matmul->multiply chain.
        lw = nc.tensor.ldweights(E_sb[:])
        keep_ld_names.add(lw.ins.name)

        # Reciprocal history (one fp16 reciprocal per norm per sample).
        rh_sb = state_pool.tile([1, NNORM * BC], DT)

        # Two broadcast-reciprocal buffers, alternated per renorm (static
        # tiles, not a rotating pool: each pool-tile allocation leaves a
        # per-queue release semaphore that serializes at program end).
        RbcA = const_pool.tile([L, BC], DT)
        RbcB = const_pool.tile([L, BC], DT)
        Rbcs = [RbcA, RbcB]

        # Two state rings: ring(j) = j%2 holds w_{32j+1..32j+32} in slots 0..31.
        WA = state_pool.tile([L, CAPB * BC], DT)
        WB = state_pool.tile([L, CAPB * BC], DT)
        rings = [WA, WB]
        # w_0 = onehot(START=0) lives at ring 1, slot 31.
        nc.vector.memset(WB[:, (CAPB - 1) * BC:CAPB * BC], 0.0)
        nc.vector.memset(WB[0:1, (CAPB - 1) * BC:CAPB * BC], 1.0)

        def wslot(t):
            """AP of w_t (full BC columns)."""
            ring = rings[((t - 1) // CAPB) % 2]
            s = (t - 1) % CAPB
            return ring[:, s * BC:(s + 1) * BC]

        ex_tiles = {}   # granule index -> ex tile (CAPB steps each)
        pend_R = None   # (broadcast reciprocal sbuf tile, application step)
        pend_rcp = None  # deferred second half of the renorm reciprocal

        # First chunk split small so step 0 starts ~9us earlier.
        chunk_steps = [CAPB, CH - CAPB] + [CH] * (T // CH - 1)
        chunk_t0 = np.cumsum([0] + chunk_steps[:-1]).tolist()
        for cs, ct0 in zip(chunk_steps, chunk_t0):
            xt = xin_pool.tile([L, cs * BC], F32, tag="xt")
            if ct0 == 0:
                # Finer DMA granularity so step 0 starts after just the first
                # few steps of x have landed (subtile deps gate per-slice).
                for q0, qn in ((0, 4), (4, 4), (8, 8), (16, 16)):
                    nc.sync.dma_start(
                        xt[:, q0 * BC:(q0 + qn) * BC],
                        xT[:, q0 * BC:(q0 + qn) * BC])
            else:
                nc.sync.dma_start(xt[:], xT[:, ct0 * BC:(ct0 + cs) * BC])
            for jj in range(cs // CAPB):
                j = (ct0 // CAPB) + jj      # capture block index
                jo = jj                      # granule offset within chunk
                ex = ex_pool.tile([L, CAPB * BC], DT)
                # Finer exp granules at startup so step 0 begins after ~4
                # steps of x instead of a full 32-step block.
                grans = ((0, 4), (4, 4), (8, 8), (16, 16)) if j == 0 \
                    else ((0, CAPB),)
                for q0, qn in grans:
                    sl = slice((jo * CAPB + q0) * BC,
                               (jo * CAPB + q0 + qn) * BC)
                    esl = slice(q0 * BC, (q0 + qn) * BC)
                    nc.scalar.activation(
                        ex[:, esl], xt[:, sl],
                        mybir.ActivationFunctionType.Exp, bias=nbias[:],
                    )
                ex_tiles[j] = ex
                for i in range(CAPB):
                    t = j * CAPB + i
                    # Apply a pending renorm to this step's ex slice (one op;
                    # same-queue ordering keeps it off the serial chain).
                    if pend_R is not None and pend_R[1] == t:
                        R = pend_R[0]
                        sl = slice(i * BC, (i + 1) * BC)
                        nc.vector.tensor_mul(ex[:, sl], ex[:, sl], R[:])
                        pend_R = None
                    # Second half of the renorm reciprocal, one step after the
                    # first ([1,16] pieces fit the per-step DVE slack), then
                    # the Pool broadcast of the completed [1,32] row.
                    if pend_rcp is not None:
                        out_ap, in_ap, m_, app_t = pend_rcp
                        nc.vector.reciprocal(out_ap, in_ap)
                        Rbc = Rbcs[m_ % 2]
                        nc.gpsimd.partition_broadcast(
                            Rbc[:], rh_sb[0:1, m_ * BC:(m_ + 1) * BC])
                        pend_R = (Rbc, app_t)
                        pend_rcp = None
                    src = wslot(t)
                    dst = wslot(t + 1)
                    for g in range(G):
                        lo, hi = GB[g], GB[g + 1]
                        P = ps_pool.tile([L, hi - lo], F32, tag=f"P{g}")
                        mm = nc.tensor.matmul(P[:], E_sb[:], src[:, lo:hi],
                                              start=True, stop=True)
                        mm.ins.ldweights = False
                        nc.vector.tensor_mul(dst[:, lo:hi],
                                             ex[:, i * BC + lo:i * BC + hi],
                                             P[:])
                    # Renorm trigger: tau = t = K(m+1); normalizer = the fp16
                    # state row 0 just written (any per-column scale works;
                    # the host uses the recorded fp16 reciprocal exactly).
                    # Broadcast it on Pool and fold into the ex slice of step
                    # tau+D, off the serial matmul/multiply chain.
                    if t % K == 0 and t > 0 and t + D <= T - 1:
                        m = t // K - 1
                        nc.vector.reciprocal(rh_sb[0:1, m * BC:m * BC + GS],
                                             dst[0:1, 0:GS])
                        pend_rcp = (rh_sb[0:1, m * BC + GS:(m + 1) * BC],
                                    dst[0:1, GS:BC], m, t + D)
                # Capture row 127 of the finished ring (w_{32j+1..32j+32});
                # the double ring gives this DMA 32 steps of WAR slack.
                ring = rings[j % 2]
                nc.sync.dma_start(
                    hist[0:1, j * CAPB * BC:(j + 1) * CAPB * BC],
                    ring[127:128, :])
                if j - 2 in ex_tiles:
                    del ex_tiles[j - 2]

        # Final (E @ w_512)[127] for samples with len == T.
        Pf = ps_pool.tile([L, BC], F32, tag="P0")
        mmf = nc.tensor.matmul(Pf[:], E_sb[:], wslot(T), start=True, stop=True)
        mmf.ins.ldweights = False
        capf = state_pool.tile([L, BC], DT)
        nc.vector.tensor_copy(capf[:], Pf[:])
        nc.sync.dma_start(hist[0:1, T * BC:T * BC + BC], capf[127:128, :])
        nc.sync.dma_start(rhist[0:1, :], rh_sb[:])

    # Tile re-pairs every matmul with its own InstLdweights regardless of the
    # non-self-loading flag; strip those (keeping the single explicit load).
    # They carry no semaphore waits (verified: all waits live on the matmuls),
    # so deleting them does not disturb synchronization.
    removed = 0
    for fn in nc.m.functions:
        for bb in fn.blocks:
            insts = bb.instructions
            for i in range(len(insts) - 1, -1, -1):
                inst = insts[i]
                if (type(inst).__name__ == "InstLdweights"
                        and inst.name not in keep_ld_names):
                    si = inst.sync_info
                    assert si is None or (not si.on_wait and not si.on_update), \
                        f"ldweights {inst.name} carries sync; refusing to drop"
                    del insts[i]
                    removed += 1
    assert removed == T * G + 1, \
        f"expected {T * G + 1} redundant ldweights, got {removed}"

    nc.compile()
    return nc


def _get_nc():
    global _CACHED_NC
    if _CACHED_NC is None:
        _CACHED_NC = _build_bass()
    return _CACHED_NC


def run_on_device(x, transit_matrix, **spmd_kwargs):
    """Shard inputs, run the SPMD kernel on 8 cores, return per-core results."""
    xT = np.ascontiguousarray(np.asarray(x, np.float32).transpose(2, 1, 0))
    trT = np.ascontiguousarray(np.asarray(transit_matrix, np.float32).T)
    in_maps = []
    for c in range(NCORES):
        xc = np.ascontiguousarray(xT[:, :, c * BC:(c + 1) * BC]).reshape(L, T * BC)
        in_maps.append({"xT": xc, "trT": trT})
    nc = _get_nc()
    return bass_utils.run_bass_kernel_spmd(
        nc, in_maps, core_ids=list(range(NCORES)), **spmd_kwargs
    )


def finish_on_host(results, x, lengths):
    """Reconstruct alpha[b] in float64 from the device captures.

    fv_t = ln(w_t) + t*SCALE_LN + sum of ln(s_m) over norms applied before t
    (norm m: s_m = 1/r_m, r_m recorded; applied at step a_m = K(m+1)+D).
    For len < T the capture is w_{len+1}[127] = exp(x[b,len,127])/256 *
    (E @ w_len)[127] (with the step-len renorm folded in when a_m == len), which
    collapses to the uniform formula below; for len == T the tail capture is
    (E @ w_512)[127] directly.
    """
    lengths = np.asarray(lengths).astype(np.int64)
    x = np.asarray(x)
    alpha = np.empty(B, np.float64)
    for c in range(NCORES):
        hist = results[c]["hist"].reshape(-1).astype(np.float64)
        rh = results[c]["rhist"].reshape(-1).astype(np.float64)
        lnS = -np.log(rh.reshape(NNORM, BC))          # ln s_m per norm m
        cum = np.zeros((NNORM + 1, BC))
        cum[1:] = np.cumsum(lnS, axis=0)
        hist_blk = hist[:T * BC].reshape(T, BC)       # hist_blk[t-1] = w_t[127]
        cap512 = hist[T * BC:]

        ln = lengths[c * BC:(c + 1) * BC]             # (BC,)
        bi = np.arange(BC)
        full = ln == T
        nf = ~full
        out = np.empty(BC, np.float64)
        # Captures that underflowed deep into fp16 subnormals lose log
        # accuracy; flag them (NaN) for the exact host fallback in kernel().
        with np.errstate(divide="ignore", invalid="ignore"):
            out[full] = T * SCALE_LN + cum[NNORM, bi[full]] + np.log(
                np.where(cap512[full] < 3e-7, np.nan, cap512[full]))
        cap = hist_blk[ln[nf], bi[nf]]                # w_{len+1}[127]
        cap = np.where(cap < 3e-7, np.nan, cap)
        x127 = x[c * BC + bi[nf], ln[nf], 127].astype(np.float64)
        # norms applied at a_m = K(m+1)+D <= len: count = (len-D)//K, clipped
        nidx = np.clip((ln[nf] - D) // K, 0, NNORM)
        with np.errstate(divide="ignore", invalid="ignore"):
            out[nf] = (np.log(cap) - x127 + (ln[nf] + 1) * SCALE_LN
                       + cum[nidx, bi[nf]])
        alpha[c * BC:(c + 1) * BC] = out
    return alpha.astype(np.float32)


def _crf_alpha_single(xb, tr, length):
    """Exact single-sample CRF forward in float64 (rare-fallback path)."""
    NEG = -10000.0
    trd = np.asarray(tr, np.float64)
    fv = np.full(L, NEG)
    fv[0] = 0.0
    for t in range(int(length)):
        sc = trd + fv[None, :] + np.asarray(xb[t], np.float64)[:, None]
        m = sc.max(axis=1)
        fv = m + np.log(np.exp(sc - m[:, None]).sum(axis=1))
    term = fv + trd[L - 1]
    m = term.max()
    return m + np.log(np.exp(term - m).sum())


def kernel(x, transit_matrix, lengths):
    x = np.asarray(x, np.float32)
    assert x.shape == (B, T, L), x.shape
    res = run_on_device(x, transit_matrix)
    alpha = finish_on_host(res.results, x, lengths)
    # fp16 captures can in principle underflow to subnormal/zero for extreme
    # samples; recompute those few (if any) exactly on host.
    bad = ~np.isfinite(alpha)
    if bad.any():
        ln = np.asarray(lengths).astype(np.int64)
        for b in np.nonzero(bad)[0]:
            alpha[b] = _crf_alpha_single(x[b], transit_matrix, ln[b])
    return alpha


# revision 44
# speedup vs baseline: 1.0680x; 1.0237x over previous
"""CRF forward-algorithm kernel for Trainium2 (8 NeuronCores, data-parallel over batch).

Math: the reference computes, per sample b,
    fv_{t+1}[next] = x_t[next] + logsumexp_prev(transit[next, prev] + fv_t[prev])   (t < len_b)
    alpha[b] = logsumexp_i(fv_{len_b}[i] + transit[STOP, i])

In linear space with E = exp(transit) this is
    w_{t+1} = exp(x_t) * (E @ w_t),      fv_t = log(w_t) + c_t
so each timestep is one fp16 128x128 @ 128x32 matmul (PE) plus one elementwise
multiply.  The 512-step chain is serial, so wall time = 512 x round latency
(PE -> PSUM -> multiply -> SBUF -> PE).  The round is minimized by:
  * loading E into the PE array once (ldweights) and marking every step matmul
    non-self-loading, which removes the ~100ns weight reload that otherwise
    sits after the semaphore wait on the critical path;
  * splitting the 32 batch columns into two groups whose multiplies run
    concurrently on the Vector (DVE) and Pool engines, each reading its own
    PSUM bank, so per-round engine time is halved.
exp(x) is pre-scaled by 1/256 and the state is renormalized every K=8 steps:
the normalizer is the fp16 state row 0 (already in SBUF), its reciprocal is
recorded for the host, broadcast across partitions on Pool, and folded into
the exp(x) slice of step tau+D in each engine's own queue order, so the renorm
never adds semaphores to the serial chain.  Because alpha needs
(E @ w_len)[STOP] and STOP = 127 is the last row of E, the per-step capture is
just row 127 of the state; the state lives in two alternating 32-slot rings so
row 127 of a finished ring is DMA'd out with 32 steps of WAR slack.  The final
log/gather bookkeeping (O(B*T) scalar work) runs on host in float64 from the
captures.
"""

import sys

sys.path.insert(0, "/opt/trn_rl_repo")

import numpy as np
from contextlib import ExitStack

import concourse.bass as bass
import concourse.tile as tile
import concourse.mybir as mybir
from concourse import bacc, bass_utils



# Problem constants (hardcoded per contract).
B, T, L = 256, 512, 128
NCORES = 8
BC = B // NCORES          # 32 samples per core
K = 8                     # renormalization period
D = 6                     # renorm application delay (steps after tau)
QW = 8                    # renorm reciprocal piece width (columns)
CAPB = 32                 # capture block (ring size)
CH = 128                  # x chunk length in timesteps
NCAP = T // CAPB          # capture blocks
NNORM = 63                # norms m=0..62: tau=8(m+1)<=504, applied at tau+D
G = 3                     # batch groups (all multiplies on DVE)
GB = [0, 11, 22, 32]      # group column boundaries
GS = BC // 2              # renorm reciprocal half width
SCALE_LN = float(np.log(256.0))
F32 = mybir.dt.float32
DT = mybir.dt.float16     # state/weights dtype

_CACHED_NC = None





def _build_bass():
    """Build the single-core Bass program (shared SPMD across 8 cores)."""
    nc = bacc.Bacc("TRN2", debug=False)

    xT = nc.dram_tensor("xT", [L, T * BC], F32, kind="ExternalInput").ap()
    trT = nc.dram_tensor("trT", [L, L], F32, kind="ExternalInput").ap()
    # hist[j*CAPB*BC + s*BC + b] = w_{32j+1+s}[127, b]; tail BC entries are
    # (E @ w_512)[127].
    hist = nc.dram_tensor("hist", [1, T * BC + BC], DT, kind="ExternalOutput").ap()
    rhist = nc.dram_tensor("rhist", [1, NNORM * BC], DT, kind="ExternalOutput").ap()

    keep_ld_names = set()
    with tile.TileContext(nc) as tc, ExitStack() as ctx, \
            nc.allow_low_precision(reason="fp16 state validated against f64 ref"):
        # One static pool for constants/state/renorm tiles (fewer pools ->
        # shorter serial event-semaphore teardown at program end).
        const_pool = ctx.enter_context(tc.tile_pool(name="const", bufs=1))
        state_pool = const_pool
        rbc_pool = const_pool
        xin_pool = ctx.enter_context(tc.tile_pool(name="xin", bufs=2))
        ex_pool = ctx.enter_context(tc.tile_pool(name="ex", bufs=3))
        ps_pool = ctx.enter_context(tc.tile_pool(name="ps", bufs=2, space="PSUM"))

        # Constants.
        nbias = const_pool.tile([L, 1], F32)
        nc.vector.memset(nbias[:], -SCALE_LN)

        tr_sb = const_pool.tile([L, L], F32)
        nc.sync.dma_start(tr_sb[:], trT[:, :])
        E_sb = const_pool.tile([L, L], DT)
        nc.scalar.activation(E_sb[:], tr_sb[:], mybir.ActivationFunctionType.Exp)
        # E is loaded into the PE array exactly once and stays resident for
        # the whole chain: every matmul below is flagged non-self-loading and
        # the redundant per-matmul InstLdweights that Tile re-inserts are
        # deleted from the module after the TileContext exits (they carry no
        # semaphore waits, so removal is sync-safe).  This takes the ~100ns
        # 128-row weight reload off the serial matmul->multiply chain.
        lw = nc.tensor.ldweights(E_sb[:])
        keep_ld_names.add(lw.ins.name)

        # Reciprocal history (one fp16 reciprocal per norm per sample).
        rh_sb = state_pool.tile([1, NNORM * BC], DT)

        # Two broadcast-reciprocal buffers, alternated per renorm (static
        # tiles, not a rotating pool: each pool-tile allocation leaves a
        # per-queue release semaphore that serializes at program end).
        RbcA = const_pool.tile([L, BC], DT)
        RbcB = const_pool.tile([L, BC], DT)
        Rbcs = [RbcA, RbcB]

        # Two state rings: ring(j) = j%2 holds w_{32j+1..32j+32} in slots 0..31.
        WA = state_pool.tile([L, CAPB * BC], DT)
        WB = state_pool.tile([L, CAPB * BC], DT)
        rings = [WA, WB]
        # w_0 = onehot(START=0) lives at ring 1, slot 31.
        nc.vector.memset(WB[:, (CAPB - 1) * BC:CAPB * BC], 0.0)
        nc.vector.memset(WB[0:1, (CAPB - 1) * BC:CAPB * BC], 1.0)

        def wslot(t):
            """AP of w_t (full BC columns)."""
            ring = rings[((t - 1) // CAPB) % 2]
            s = (t - 1) % CAPB
            return ring[:, s * BC:(s + 1) * BC]

        ex_tiles = {}   # granule index -> ex tile (CAPB steps each)
        pend_R = None   # (broadcast reciprocal sbuf tile, application step)
        pend_rcp = None  # deferred second half of the renorm reciprocal

        # First chunk split small so step 0 starts ~9us earlier.
        chunk_steps = [CAPB, CH - CAPB] + [CH] * (T // CH - 1)
        chunk_t0 = np.cumsum([0] + chunk_steps[:-1]).tolist()
        for cs, ct0 in zip(chunk_steps, chunk_t0):
            xt = xin_pool.tile([L, cs * BC], F32, tag="xt")
            if ct0 == 0:
                # Finer DMA granularity so step 0 starts after just the first
                # few steps of x have landed (subtile deps gate per-slice).
                for q0, qn in ((0, 4), (4, 4), (8, 8), (16, 16)):
                    nc.sync.dma_start(
                        xt[:, q0 * BC:(q0 + qn) * BC],
                        xT[:, q0 * BC:(q0 + qn) * BC])
            else:
                nc.sync.dma_start(xt[:], xT[:, ct0 * BC:(ct0 + cs) * BC])
            for jj in range(cs // CAPB):
                j = (ct0 // CAPB) + jj      # capture block index
                jo = jj                      # granule offset within chunk
                ex = ex_pool.tile([L, CAPB * BC], DT)
                # Finer exp granules at startup so step 0 begins after ~4
                # steps of x instead of a full 32-step block.
                grans = ((0, 4), (4, 4), (8, 8), (16, 16)) if j == 0 \
                    else ((0, CAPB),)
                for q0, qn in grans:
                    sl = slice((jo * CAPB + q0) * BC,
                               (jo * CAPB + q0 + qn) * BC)
                    esl = slice(q0 * BC, (q0 + qn) * BC)
                    nc.scalar.activation(
                        ex[:, esl], xt[:, sl],
                        mybir.ActivationFunctionType.Exp, bias=nbias[:],
                    )
                ex_tiles[j] = ex
                for i in range(CAPB):
                    t = j * CAPB + i
                    # Apply a pending renorm to this step's ex slice (one op;
                    # same-queue ordering keeps it off the serial chain).
                    if pend_R is not None and pend_R[1] == t:
                        R = pend_R[0]
                        sl = slice(i * BC, (i + 1) * BC)
                        nc.vector.tensor_mul(ex[:, sl], ex[:, sl], R[:])
                        pend_R = None
                    # Next [1,8] piece of a pending renorm reciprocal (one
                    # piece per step fits the per-step DVE slack); after the
                    # last piece, the Pool broadcast of the completed row.
                    if pend_rcp:
                        q, m_, nsrc, app_t = pend_rcp.pop(0)
                        nc.vector.reciprocal(
                            rh_sb[0:1, m_ * BC + q * QW:m_ * BC + (q + 1) * QW],
                            nsrc[0:1, q * QW:(q + 1) * QW])
                        if not pend_rcp:
                            Rbc = Rbcs[m_ % 2]
                            nc.gpsimd.partition_broadcast(
                                Rbc[:], rh_sb[0:1, m_ * BC:(m_ + 1) * BC])
                            pend_R = (Rbc, app_t)
                    src = wslot(t)
                    dst = wslot(t + 1)
                    for g in range(G):
                        lo, hi = GB[g], GB[g + 1]
                        P = ps_pool.tile([L, hi - lo], F32, tag=f"P{g}")
                        mm = nc.tensor.matmul(P[:], E_sb[:], src[:, lo:hi],
                                              start=True, stop=True)
                        mm.ins.ldweights = False
                        nc.vector.tensor_mul(dst[:, lo:hi],
                                             ex[:, i * BC + lo:i * BC + hi],
                                             P[:])
                    # Renorm trigger: tau = t = K(m+1); normalizer = the fp16
                    # state row 0 just written (any per-column scale works;
                    # the host uses the recorded fp16 reciprocal exactly).
                    # Broadcast it on Pool and fold into the ex slice of step
                    # tau+D, off the serial matmul/multiply chain.
                    if t % K == 0 and t > 0 and t + D <= T - 1:
                        m = t // K - 1
                        pend_rcp = [(q, m, dst, t + D)
                                    for q in range(BC // QW)]
                # Capture row 127 of the finished ring (w_{32j+1..32j+32});
                # the double ring gives this DMA 32 steps of WAR slack.
                ring = rings[j % 2]
                nc.sync.dma_start(
                    hist[0:1, j * CAPB * BC:(j + 1) * CAPB * BC],
                    ring[127:128, :])
                if j - 2 in ex_tiles:
                    del ex_tiles[j - 2]

        # Final (E @ w_512)[127] for samples with len == T.
        Pf = ps_pool.tile([L, BC], F32, tag="P0")
        mmf = nc.tensor.matmul(Pf[:], E_sb[:], wslot(T), start=True, stop=True)
        mmf.ins.ldweights = False
        capf = state_pool.tile([L, BC], DT)
        nc.vector.tensor_copy(capf[:], Pf[:])
        nc.sync.dma_start(hist[0:1, T * BC:T * BC + BC], capf[127:128, :])
        nc.sync.dma_start(rhist[0:1, :], rh_sb[:])

    # Tile re-pairs every matmul with its own InstLdweights regardless of the
    # non-self-loading flag; strip those (keeping the single explicit load).
    # They carry no semaphore waits (verified: all waits live on the matmuls),
    # so deleting them does not disturb synchronization.
    removed = 0
    for fn in nc.m.functions:
        for bb in fn.blocks:
            insts = bb.instructions
            for i in range(len(insts) - 1, -1, -1):
                inst = insts[i]
                if (type(inst).__name__ == "InstLdweights"
                        and inst.name not in keep_ld_names):
                    si = inst.sync_info
                    assert si is None or (not si.on_wait and not si.on_update), \
                        f"ldweights {inst.name} carries sync; refusing to drop"
                    del insts[i]
                    removed += 1
    assert removed == T * G + 1, \
        f"expected {T * G + 1} redundant ldweights, got {removed}"

    nc.compile()
    return nc


def _get_nc():
    global _CACHED_NC
    if _CACHED_NC is None:
        _CACHED_NC = _build_bass()
    return _CACHED_NC


def run_on_device(x, transit_matrix, **spmd_kwargs):
    """Shard inputs, run the SPMD kernel on 8 cores, return per-core results."""
    xT = np.ascontiguousarray(np.asarray(x, np.float32).transpose(2, 1, 0))
    trT = np.ascontiguousarray(np.asarray(transit_matrix, np.float32).T)
    in_maps = []
    for c in range(NCORES):
        xc = np.ascontiguousarray(xT[:, :, c * BC:(c + 1) * BC]).reshape(L, T * BC)
        in_maps.append({"xT": xc, "trT": trT})
    nc = _get_nc()
    return bass_utils.run_bass_kernel_spmd(
        nc, in_maps, core_ids=list(range(NCORES)), **spmd_kwargs
    )


def finish_on_host(results, x, lengths):
    """Reconstruct alpha[b] in float64 from the device captures.

    fv_t = ln(w_t) + t*SCALE_LN + sum of ln(s_m) over norms applied before t
    (norm m: s_m = 1/r_m, r_m recorded; applied at step a_m = K(m+1)+D).
    For len < T the capture is w_{len+1}[127] = exp(x[b,len,127])/256 *
    (E @ w_len)[127] (with the step-len renorm folded in when a_m == len), which
    collapses to the uniform formula below; for len == T the tail capture is
    (E @ w_512)[127] directly.
    """
    lengths = np.asarray(lengths).astype(np.int64)
    x = np.asarray(x)
    alpha = np.empty(B, np.float64)
    for c in range(NCORES):
        hist = results[c]["hist"].reshape(-1).astype(np.float64)
        rh = results[c]["rhist"].reshape(-1).astype(np.float64)
        lnS = -np.log(rh.reshape(NNORM, BC))          # ln s_m per norm m
        cum = np.zeros((NNORM + 1, BC))
        cum[1:] = np.cumsum(lnS, axis=0)
        hist_blk = hist[:T * BC].reshape(T, BC)       # hist_blk[t-1] = w_t[127]
        cap512 = hist[T * BC:]

        ln = lengths[c * BC:(c + 1) * BC]             # (BC,)
        bi = np.arange(BC)
        full = ln == T
        nf = ~full
        out = np.empty(BC, np.float64)
        # Captures that underflowed deep into fp16 subnormals lose log
        # accuracy; flag them (NaN) for the exact host fallback in kernel().
        with np.errstate(divide="ignore", invalid="ignore"):
            out[full] = T * SCALE_LN + cum[NNORM, bi[full]] + np.log(
                np.where(cap512[full] < 3e-7, np.nan, cap512[full]))
        cap = hist_blk[ln[nf], bi[nf]]                # w_{len+1}[127]
        cap = np.where(cap < 3e-7, np.nan, cap)
        x127 = x[c * BC + bi[nf], ln[nf], 127].astype(np.float64)
        # norms applied at a_m = K(m+1)+D <= len: count = (len-D)//K, clipped
        nidx = np.clip((ln[nf] - D) // K, 0, NNORM)
        with np.errstate(divide="ignore", invalid="ignore"):
            out[nf] = (np.log(cap) - x127 + (ln[nf] + 1) * SCALE_LN
                       + cum[nidx, bi[nf]])
        alpha[c * BC:(c + 1) * BC] = out
    return alpha.astype(np.float32)


def _crf_alpha_single(xb, tr, length):
    """Exact single-sample CRF forward in float64 (rare-fallback path)."""
    NEG = -10000.0
    trd = np.asarray(tr, np.float64)
    fv = np.full(L, NEG)
    fv[0] = 0.0
    for t in range(int(length)):
        sc = trd + fv[None, :] + np.asarray(xb[t], np.float64)[:, None]
        m = sc.max(axis=1)
        fv = m + np.log(np.exp(sc - m[:, None]).sum(axis=1))
    term = fv + trd[L - 1]
    m = term.max()
    return m + np.log(np.exp(term - m).sum())


def kernel(x, transit_matrix, lengths):
    x = np.asarray(x, np.float32)
    assert x.shape == (B, T, L), x.shape
    res = run_on_device(x, transit_matrix)
    alpha = finish_on_host(res.results, x, lengths)
    # fp16 captures can in principle underflow to subnormal/zero for extreme
    # samples; recompute those few (if any) exactly on host.
    bad = ~np.isfinite(alpha)
    if bad.any():
        ln = np.asarray(lengths).astype(np.int64)
        for b in np.nonzero(bad)[0]:
            alpha[b] = _crf_alpha_single(x[b], transit_matrix, ln[b])
    return alpha
